# revision 6
# baseline (speedup 1.0000x reference)
"""MiniCPM attention block on 8 Trainium2 NeuronCores.

Sharding: core c handles batch b = c // 4 and the 8 heads
[ (c%4)*8, (c%4)*8 + 8 ) of that batch (tensor-parallel over heads +
data-parallel over batch).  Each core computes a partial output
x @ block-of-Wo.T of shape [S, HID]; the host sums the 4 partials per
batch.  No collectives.

Device pipeline per core (S=2048 tokens, 8 heads of d=64):
  1. per 512-token chunk: qT/kT = (x @ Wq_s.T).T with RoPE fused into
     the PSUM->SBUF evict, and v = x @ Wv_s.T (stored [tk, 8*65] with a
     ones column per head) — one x load per chunk shared by Q/K/V, all
     12 PSUM groups rotating through one 8-slot pool.
  2. per head-pair, per tq-chunk j (512): S.T tiles [tk 128, tq 512]
     on PE (2 heads packed via row tiling -> concurrent), exp on ACT
     (pair-fused into one [128,1024] op), causal zeroing via gpsimd
     affine_select, AV accumulation on PE with the ones column giving
     the softmax denominator for free.  Normalize via DVE reciprocal +
     a K=1 PE broadcast matmul (ones^T @ rec -> psum rows 64:128) +
     DVE copy/mul — no DRAM bounce.  The i-loop is software-pipelined
     (S two tiles ahead of AV) so PE never waits on the ACT exp
     round-trip.  Causal boundary tiles compute only the valid tq
     suffix.
  3. out_partial = attn_outT.T @ Wo_s.T in [128,512] PSUM groups with
     DVE evicts, interleaved per tq-column so PE has dense work while
     ACT runs the next column's exps.

Self-contained: shapes hardcoded from the problem spec.
"""
import numpy as np
import ml_dtypes

S = 2048
HID = 2048
NH = 32
DH = 64
N_CORES = 8
HEADS_PER_CORE = NH // (N_CORES // 2)   # 8
BLK = HEADS_PER_CORE * DH               # 512
ROPE_BASE = 10000.0

_PROGRAMS = {}


def _rope_cache():
    inv_freq = 1.0 / (ROPE_BASE ** (np.arange(0, DH, 2, dtype=np.float32) / DH))
    t = np.arange(S, dtype=np.float32)
    freqs = np.outer(t, inv_freq)                     # [S, 32]
    emb = np.concatenate([freqs, freqs], axis=-1)     # [S, 64]
    return np.cos(emb), np.sin(emb)


def build_program(variant="causal"):
    """Build the Bacc program (one NEFF, run SPMD on 8 cores)."""
    import concourse.bass as bass
    import concourse.mybir as mybir
    import concourse.tile as tile
    from concourse import bacc

    fp32 = mybir.dt.float32
    f32r = mybir.dt.float32r
    DT = mybir.dt.bfloat16      # dram/lhs/rhs matmul dtype
    SDT = mybir.dt.bfloat16     # on-chip storage for q/k/v/p/attn
    CH = 512                    # token chunk for phase 1

    def mm(ap):
        return ap.bitcast(DT) if DT is not ap.dtype else ap

    causal = variant == "causal"
    NCH = S // CH            # phase-1 token chunks
    NT = S // 128            # 16 token tiles
    NPR = 4                  # head pairs

    nc = bacc.Bacc("TRN2", target_bir_lowering=False, debug=False,
                   enable_asserts=False, num_devices=N_CORES)

    xT = nc.dram_tensor("xT", [HID, S], DT, kind="ExternalInput").ap()
    wqT = nc.dram_tensor("wqT", [HID, BLK], DT, kind="ExternalInput").ap()
    wkT = nc.dram_tensor("wkT", [HID, BLK], DT, kind="ExternalInput").ap()
    wvT = nc.dram_tensor("wvT", [HID, BLK], DT, kind="ExternalInput").ap()
    woT = nc.dram_tensor("woT", [BLK, HID], DT, kind="ExternalInput").ap()
    cos2 = nc.dram_tensor("cos2", [128, S], fp32, kind="ExternalInput").ap()
    sin2s = nc.dram_tensor("sin2s", [128, S], fp32, kind="ExternalInput").ap()
    if not causal:
        maskT = nc.dram_tensor("maskT", [S, S], mybir.dt.bfloat16,
                               kind="ExternalInput").ap()
    out = nc.dram_tensor("out", [S, HID], fp32, kind="ExternalOutput").ap()

    tc_ctx = tile.TileContext(nc)

    def phase1(tc, qT_sb, kT_sb, v_sb):
        with tc.tile_pool(name="consts", bufs=1) as cpool, \
             tc.tile_pool(name="wall", bufs=1) as wpool, \
             tc.tile_pool(name="xa", bufs=2) as xa, \
             tc.tile_pool(name="ropetmp", bufs=2) as rt, \
             tc.tile_pool(name="ps1", bufs=8, space="PSUM") as ps1:
            # DMA issue order matters for startup latency: the first
            # matmul group needs wq + x chunk 0; cos/sin only matter
            # ~a dozen matmuls later, wk/wv later still.
            wq_sb = wpool.tile([128, 16, BLK], DT)
            wk_sb = wpool.tile([128, 16, BLK], DT)
            wv_sb = wpool.tile([128, 16, BLK], DT)
            wqd = wqT.rearrange("(k p) m -> p k m", p=128)
            for kg in range(4):
                nc.sync.dma_start(out=wq_sb[:, 4 * kg:4 * kg + 4, :],
                                  in_=wqd[:, 4 * kg:4 * kg + 4, :])
            cos_sb = cpool.tile([128, S], fp32)
            sin_sb = cpool.tile([128, S], fp32)
            nc.sync.dma_start(out=cos_sb, in_=cos2)
            nc.sync.dma_start(out=sin_sb, in_=sin2s)
            for w_sb, w_d in ((wk_sb, wkT), (wv_sb, wvT)):
                wd = w_d.rearrange("(k p) m -> p k m", p=128)
                for kg in range(4):
                    nc.sync.dma_start(out=w_sb[:, 4 * kg:4 * kg + 4, :],
                                      in_=wd[:, 4 * kg:4 * kg + 4, :])

            for n in range(NCH):
                sl = slice(n * CH, (n + 1) * CH)
                x_ch = xa.tile([128, 16, CH], DT, name=f"x_{n}", tag="x")
                xd = xT[:, sl].rearrange("(k p) t -> p k t", p=128)
                for kg in range(4):
                    nc.sync.dma_start(out=x_ch[:, 4 * kg:4 * kg + 4, :],
                                      in_=xd[:, 4 * kg:4 * kg + 4, :])
                # --- Q/K + RoPE ---
                for w_sb, dst, wn in ((wq_sb, qT_sb, "q"), (wk_sb, kT_sb, "k")):
                    for m in range(NPR):
                        ps = ps1.tile([128, CH], fp32,
                                      name=f"ps{wn}{m}_{n}", tag="ps1")
                        for k in range(16):
                            nc.tensor.matmul(
                                ps,
                                lhsT=mm(w_sb[:, k, m * 128:(m + 1) * 128]),
                                rhs=mm(x_ch[:, k, :]),
                                start=(k == 0), stop=(k == 15))
                        # rot = partition-shift of ps via 4 ACT copies
                        # (PSUM source, so the SBUF same-start rule only
                        # sees the output), then 2 full-width DVE muls +
                        # GPS add.  Spreads rope across ACT/DVE/GPS.
                        rot = rt.tile([128, CH], fp32, name=f"ro{wn}{m}_{n}", tag="ro")
                        for (d, s_) in ((0, 32), (32, 0), (64, 96), (96, 64)):
                            nc.scalar.copy(rot[d:d + 32, :], ps[s_:s_ + 32, :])
                        t1 = rt.tile([128, CH], fp32, name=f"t1{wn}{m}_{n}", tag="t1")
                        t2 = rt.tile([128, CH], fp32, name=f"t2{wn}{m}_{n}", tag="t2")
                        nc.vector.tensor_mul(t1, ps, cos_sb[:, sl])
                        nc.vector.tensor_mul(t2, rot, sin_sb[:, sl])
                        nc.gpsimd.tensor_add(dst[:, m, sl], t1, t2)
                # --- V (same x chunk) ---
                for s_ in range(CH // 128):
                    t16 = (n * CH) // 128 + s_
                    ps = ps1.tile([128, BLK], fp32, name=f"psv{t16}", tag="ps1")
                    for k in range(16):
                        nc.tensor.matmul(
                            ps,
                            lhsT=mm(x_ch[:, k, s_ * 128:(s_ + 1) * 128]),
                            rhs=mm(wv_sb[:, k, :]),
                            start=(k == 0), stop=(k == 15))
                    dstv = v_sb[:, t16, :].rearrange("p (h c) -> p h c", c=65)[:, :, 0:64]
                    nc.vector.tensor_copy(dstv, ps.rearrange("p (h c) -> p h c", c=64))

    with tc_ctx as tc:
        with tc.tile_pool(name="qk_sb", bufs=1) as qk_pool, \
             tc.tile_pool(name="v_sb_pool", bufs=1) as v_pool:
            qT_sb = qk_pool.tile([128, NPR, S], SDT)
            kT_sb = qk_pool.tile([128, NPR, S], SDT)
            v_sb = v_pool.tile([128, NT, HEADS_PER_CORE * 65], SDT)
            ones_ap = v_sb.rearrange("p t (h c) -> p t h c", c=65)[:, :, :, 64:65]
            nc.vector.memset(ones_ap, 1.0)

            phase1(tc, qT_sb, kT_sb, v_sb)

            # -------- phase 2+3: attention + fused out-projection ----
            with tc.tile_pool(name="attn_pool", bufs=1) as apool, \
                 tc.tile_pool(name="wo", bufs=1) as wop, \
                 tc.tile_pool(name="onesp", bufs=1) as onesp:
                attn_sb = apool.tile([128, NPR, S], SDT)
                wo_sb = wop.tile([128, NPR, HID], DT)
                nc.sync.dma_start(out=wo_sb,
                                  in_=woT.rearrange("(r p) o -> p r o", p=128))
                ones_sb = onesp.tile([1, 64], DT)
                nc.vector.memset(ones_sb, 1.0)
                with tc.tile_pool(name="ppool", bufs=3) as ppool, \
                     tc.tile_pool(name="npool", bufs=2) as npool, \
                     tc.tile_pool(name="mpool", bufs=2) as mpool, \
                     tc.tile_pool(name="ostage", bufs=2) as ostage, \
                     tc.tile_pool(name="pss", bufs=2, space="PSUM") as pss, \
                     tc.tile_pool(name="psav", bufs=1, space="PSUM") as psav, \
                     tc.tile_pool(name="opool", bufs=2, space="PSUM") as opool:

                    from collections import deque
                    pending_out = deque()

                    def emit_outproj_group(t16, no):
                        o_ps = opool.tile([128, 512], fp32,
                                          name=f"o{t16}_{no}", tag="o")
                        for pr_ in range(NPR):
                            nc.tensor.matmul(
                                o_ps,
                                lhsT=mm(attn_sb[:, pr_,
                                                t16 * 128:(t16 + 1) * 128]),
                                rhs=mm(wo_sb[:, pr_,
                                             no * 512:(no + 1) * 512]),
                                start=(pr_ == 0), stop=(pr_ == NPR - 1))
                        o_sb = ostage.tile([128, 512], fp32,
                                           name=f"os{t16}_{no}", tag="os")
                        nc.vector.tensor_copy(o_sb, o_ps)
                        nc.sync.dma_start(
                            out=out[t16 * 128:(t16 + 1) * 128,
                                    no * 512:(no + 1) * 512],
                            in_=o_sb)

                    def drain_one():
                        if pending_out:
                            t16, no = pending_out.popleft()
                            emit_outproj_group(t16, no)

                    def attend(pr, j, mask_col):
                        n_i = 4 * j + 4 if causal else NT
                        # full-bank AV tiles: rows 0-64 accumulate
                        # (out.T | denom); rows 64-127 later hold the
                        # PE-broadcast reciprocal
                        av = [psav.tile([128, 512], fp32,
                                        name=f"av{half}_{pr}_{j}", tag=f"av{half}")
                              for half in range(2)]
                        s_t, p_t, nw_t = {}, {}, {}

                        def emit_S(i):
                            # causal boundary tiles only need the tq
                            # suffix [512j+off, 512(j+1))
                            off = max(0, 128 * (i - 4 * j)) if causal else 0
                            nw = 512 - off
                            nw_t[i] = (off, nw)
                            s_ps = pss.tile([128, 1024], fp32,
                                            name=f"s_{pr}_{j}_{i}", tag="s")
                            for half in range(2):
                                r0 = 64 * half
                                nc.tensor.matmul(
                                    s_ps[:, half * 512:half * 512 + nw],
                                    lhsT=mm(kT_sb[r0:r0 + 64, pr,
                                                  i * 128:(i + 1) * 128]),
                                    rhs=mm(qT_sb[r0:r0 + 64, pr,
                                                 j * 512 + off:(j + 1) * 512]),
                                    start=True, stop=True,
                                    tile_position=(r0, 0))
                            s_t[i] = s_ps

                        def emit_exp(i):
                            off, nw = nw_t[i]
                            s_ps = s_t[i]
                            s_v = s_ps.rearrange("q (h t) -> q h t", h=2)[:, :, 0:nw]
                            p = ppool.tile([128, 1024], SDT,
                                           name=f"p_{pr}_{j}_{i}", tag="p")
                            p_v = p.rearrange("q (h t) -> q h t", h=2)[:, :, 0:nw]
                            if causal:
                                nc.scalar.activation(p_v, s_v,
                                                     mybir.ActivationFunctionType.Exp,
                                                     scale=0.125)
                                if i >= 4 * j:
                                    # keep iff tq - tk >= 0 (base is 0 on
                                    # boundary tiles thanks to the suffix)
                                    nc.gpsimd.affine_select(
                                        out=p_v, in_=p_v,
                                        compare_op=mybir.AluOpType.is_ge,
                                        fill=0.0,
                                        base=512 * j + off - 128 * i,
                                        pattern=[[0, 2], [1, nw]],
                                        channel_multiplier=-1)
                            else:
                                tmp = ppool.tile([128, 1024], fp32,
                                                 name=f"pt_{pr}_{j}_{i}", tag="pt")
                                for half in range(2):
                                    nc.vector.scalar_tensor_tensor(
                                        out=tmp[:, half * 512:(half + 1) * 512],
                                        in0=s_ps[:, half * 512:(half + 1) * 512],
                                        scalar=0.125,
                                        in1=mask_col[:, i, :],
                                        op0=mybir.AluOpType.mult,
                                        op1=mybir.AluOpType.add)
                                nc.scalar.activation(p, tmp,
                                                     mybir.ActivationFunctionType.Exp)
                            p_t[i] = p

                        def emit_AV(i, first, last):
                            off, nw = nw_t[i]
                            p = p_t[i]
                            for half in range(2):
                                h = 2 * pr + half
                                nc.tensor.matmul(
                                    av[half][0:65, off:512],
                                    lhsT=mm(v_sb[:, i, 65 * h:65 * h + 65]),
                                    rhs=mm(p[:, half * 512:half * 512 + nw]),
                                    start=first, stop=last)

                        # software pipeline: S runs 2 tiles ahead of AV,
                        # with outproj matmul groups drained between
                        # iterations as PE filler while ACT runs exp.
                        # (ascending i is required: AV i=0 writes the full
                        # 512 with start=True, clearing has_written before
                        # the suffix-trimmed boundary tiles accumulate)
                        order = list(range(n_i))
                        first_i, last_i = order[0], order[-1]
                        emit_S(order[0])
                        if n_i > 1:
                            emit_S(order[1])
                        emit_exp(order[0])
                        for ii, i in enumerate(order):
                            if ii + 2 < n_i:
                                emit_S(order[ii + 2])
                            if ii + 1 < n_i:
                                emit_exp(order[ii + 1])
                            emit_AV(i, i == first_i, i == last_i)
                            drain_one()

                        # normalize rows 0..63 by row 64: reciprocal on
                        # DVE, partition-broadcast via a K=1 PE matmul
                        # (ones^T @ rec) into psum rows 64:128, then DVE
                        # copy + multiply into attn_sb.
                        t2 = npool.tile([128, 512], SDT,
                                        name=f"t2_{pr}_{j}", tag="t2")
                        for half in range(2):
                            r0 = 64 * half
                            rec = npool.tile([1, 512], SDT,
                                             name=f"rec{half}_{pr}_{j}",
                                             tag=f"rec{half}")
                            with nc.allow_low_precision(reason="bf16 1/denom feeds a bf16 broadcast matmul; 2^-8 rel err is far under the accuracy budget"):
                                nc.vector.reciprocal(rec, av[half][64:65, :])
                            nc.tensor.matmul(av[half][64:128, :],
                                             lhsT=ones_sb, rhs=rec,
                                             start=True, stop=True,
                                             tile_position=(0, 64))
                            nc.vector.tensor_copy(t2[r0:r0 + 64, :],
                                                  av[half][0:64, :])
                            nc.vector.tensor_mul(
                                attn_sb[r0:r0 + 64, pr, j * 512:(j + 1) * 512],
                                t2[r0:r0 + 64, :], av[half][64:128, :])

                    for j in range(4):
                        if causal:
                            mask_col = None
                        else:
                            mask_col = mpool.tile([128, NT, 512],
                                                  mybir.dt.bfloat16,
                                                  name=f"mc{j}", tag="mc")
                            nc.sync.dma_start(
                                out=mask_col,
                                in_=maskT[:, j * 512:(j + 1) * 512]
                                .rearrange("(i p) t -> p i t", p=128))
                        for pr in range(NPR):
                            attend(pr, j, mask_col)
                        # column j's attention rows are complete; queue
                        # its out-projection as PE filler for column j+1
                        for t16 in range(4 * j, 4 * j + 4):
                            for no in range(4):
                                pending_out.append((t16, no))
                    while pending_out:
                        drain_one()
    nc.compile()
    return nc


def _get_program(variant):
    if variant not in _PROGRAMS:
        _PROGRAMS[variant] = build_program(variant)
    return _PROGRAMS[variant]


def make_in_maps(hidden_states, attention_mask, position_ids, Wq, Wk, Wv, Wo,
                 variant):
    npdt = ml_dtypes.bfloat16
    cos, sin = _rope_cache()
    in_maps = []
    for c in range(N_CORES):
        b = c // (N_CORES // 2)
        hb = c % (N_CORES // 2)
        rs = slice(hb * BLK, (hb + 1) * BLK)
        pos = np.asarray(position_ids[b]).astype(np.int64)
        cos_b = cos[pos].T.astype(np.float32)     # [64, S]
        sin_b = sin[pos].T.astype(np.float32)
        sin_s = np.concatenate([-sin_b[:32], sin_b[32:]], axis=0)
        m = {
            "xT": np.ascontiguousarray(np.asarray(hidden_states)[b].T).astype(npdt),
            "wqT": np.ascontiguousarray(np.asarray(Wq)[rs].T).astype(npdt),
            "wkT": np.ascontiguousarray(np.asarray(Wk)[rs].T).astype(npdt),
            "wvT": np.ascontiguousarray(np.asarray(Wv)[rs].T).astype(npdt),
            "woT": np.ascontiguousarray(np.asarray(Wo)[:, rs].T).astype(npdt),
            "cos2": np.ascontiguousarray(np.concatenate([cos_b, cos_b], axis=0)),
            "sin2s": np.ascontiguousarray(np.concatenate([sin_s, sin_s], axis=0)),
        }
        if variant == "general":
            m["maskT"] = np.ascontiguousarray(
                np.asarray(attention_mask)[b, 0].T).astype(ml_dtypes.bfloat16)
        in_maps.append(m)
    return in_maps


def detect_causal(attention_mask):
    am = np.asarray(attention_mask)
    if am.shape != (2, 1, S, S):
        return False
    neg = np.float32(np.finfo(np.float32).min)
    canonical = np.where(np.tril(np.ones((S, S), dtype=bool)), np.float32(0.0), neg)
    return bool(np.array_equal(am[0, 0], canonical) and
                np.array_equal(am[1, 0], canonical))


def kernel(hidden_states, attention_mask, position_ids, Wq, Wk, Wv, Wo):
    hidden_states = np.asarray(hidden_states, dtype=np.float32)
    attention_mask = np.asarray(attention_mask, dtype=np.float32)
    Wq, Wk, Wv, Wo = (np.asarray(w, dtype=np.float32) for w in (Wq, Wk, Wv, Wo))

    variant = "causal" if detect_causal(attention_mask) else "general"
    nc = _get_program(variant)
    in_maps = make_in_maps(hidden_states, attention_mask, position_ids,
                           Wq, Wk, Wv, Wo, variant)

    from concourse import bass2jax
    results = bass2jax.run_bass_via_pjrt(nc, in_maps, n_cores=N_CORES)

    out = np.zeros((2, S, HID), dtype=np.float64)
    for c in range(N_CORES):
        b = c // (N_CORES // 2)
        out[b] += results[c]["out"].astype(np.float64)
    return out.astype(np.float32)


# revision 13
# speedup vs baseline: 1.1730x; 1.1730x over previous
"""MiniCPM attention block on 8 Trainium2 NeuronCores.

Sharding: core c handles batch b = c // 4 and the 8 heads
[ (c%4)*8, (c%4)*8 + 8 ) of that batch (tensor-parallel over heads +
data-parallel over batch).  Each core computes a partial output
x @ block-of-Wo.T of shape [S, HID]; the host sums the 4 partials per
batch.  No collectives.

Device pipeline per core (S=2048 tokens, 8 heads of d=64):
  1. per 512-token chunk: qT/kT = (x @ Wq_s.T).T with RoPE fused into
     the PSUM->SBUF evict, and v = x @ Wv_s.T (stored [tk, 8*65] with a
     ones column per head) — one x load per chunk shared by Q/K/V, all
     12 PSUM groups rotating through one 8-slot pool.
  2. per head-pair, per tq-chunk j (512): S.T tiles [tk 128, tq 512]
     on PE (2 heads packed via row tiling -> concurrent), exp on ACT
     (pair-fused into one [128,1024] op), causal zeroing via gpsimd
     affine_select, AV accumulation on PE with the ones column giving
     the softmax denominator for free.  Normalize via DVE reciprocal +
     a K=1 PE broadcast matmul (ones^T @ rec -> psum rows 64:128) +
     DVE copy/mul — no DRAM bounce.  The i-loop is software-pipelined
     (S two tiles ahead of AV) so PE never waits on the ACT exp
     round-trip.  Causal boundary tiles compute only the valid tq
     suffix.
  3. out_partial = attn_outT.T @ Wo_s.T in [128,512] PSUM groups with
     DVE evicts, interleaved per tq-column so PE has dense work while
     ACT runs the next column's exps.

Self-contained: shapes hardcoded from the problem spec.
"""
import numpy as np
import ml_dtypes

S = 2048
HID = 2048
NH = 32
DH = 64
N_CORES = 8
HEADS_PER_CORE = NH // (N_CORES // 2)   # 8
BLK = HEADS_PER_CORE * DH               # 512
ROPE_BASE = 10000.0

_PROGRAMS = {}


def _rope_cache():
    inv_freq = 1.0 / (ROPE_BASE ** (np.arange(0, DH, 2, dtype=np.float32) / DH))
    t = np.arange(S, dtype=np.float32)
    freqs = np.outer(t, inv_freq)                     # [S, 32]
    emb = np.concatenate([freqs, freqs], axis=-1)     # [S, 64]
    return np.cos(emb), np.sin(emb)


def build_program(variant="causal"):
    """Build the Bacc program (one NEFF, run SPMD on 8 cores)."""
    import concourse.bass as bass
    import concourse.mybir as mybir
    import concourse.tile as tile
    from concourse import bacc

    fp32 = mybir.dt.float32
    f32r = mybir.dt.float32r
    DT = mybir.dt.bfloat16      # dram/lhs/rhs matmul dtype
    SDT = mybir.dt.bfloat16     # on-chip storage for q/k/v/p/attn
    CH = 512                    # token chunk for phase 1

    def mm(ap):
        return ap.bitcast(DT) if DT is not ap.dtype else ap

    causal = variant == "causal"
    NCH = S // CH            # phase-1 token chunks
    NT = S // 128            # 16 token tiles
    NPR = 4                  # head pairs

    nc = bacc.Bacc("TRN2", target_bir_lowering=False, debug=False,
                   enable_asserts=False, num_devices=N_CORES)

    xT = nc.dram_tensor("xT", [HID, S], DT, kind="ExternalInput").ap()
    wqT = nc.dram_tensor("wqT", [HID, BLK], DT, kind="ExternalInput").ap()
    wkT = nc.dram_tensor("wkT", [HID, BLK], DT, kind="ExternalInput").ap()
    wvT = nc.dram_tensor("wvT", [HID, BLK], DT, kind="ExternalInput").ap()
    woT = nc.dram_tensor("woT", [BLK, HID], DT, kind="ExternalInput").ap()
    cos2 = nc.dram_tensor("cos2", [128, S], DT, kind="ExternalInput").ap()
    sin2s = nc.dram_tensor("sin2s", [128, S], DT, kind="ExternalInput").ap()
    if not causal:
        maskT = nc.dram_tensor("maskT", [S, S], mybir.dt.bfloat16,
                               kind="ExternalInput").ap()
    out = nc.dram_tensor("out", [S, HID], fp32, kind="ExternalOutput").ap()

    tc_ctx = tile.TileContext(nc)

    def phase1(tc, qT_sb, kT_sb, v_sb):
        with tc.tile_pool(name="consts", bufs=1) as cpool, \
             tc.tile_pool(name="wall", bufs=1) as wpool, \
             tc.tile_pool(name="xa", bufs=2) as xa, \
             tc.tile_pool(name="ropetmp", bufs=2) as rt, \
             tc.tile_pool(name="ps1", bufs=8, space="PSUM") as ps1:
            # DMA emission order matters for startup latency: the first
            # matmul group needs x chunk 0 + wq only; cos/sin only matter
            # ~a dozen matmuls later; wk/wv are emitted mid-chunk-0 so
            # they don't steal HBM bandwidth from the critical prefetch.
            wq_sb = wpool.tile([128, 16, BLK], DT)
            wk_sb = wpool.tile([128, 16, BLK], DT)
            wv_sb = wpool.tile([128, 16, BLK], DT)
            cos_sb = cpool.tile([128, S], DT)
            sin_sb = cpool.tile([128, S], DT)

            def dma_w(w_sb, w_d):
                wd = w_d.rearrange("(k p) m -> p k m", p=128)
                for kg in range(4):
                    nc.sync.dma_start(out=w_sb[:, 4 * kg:4 * kg + 4, :],
                                      in_=wd[:, 4 * kg:4 * kg + 4, :])

            x_tiles = {}

            def dma_x(n):
                sl = slice(n * CH, (n + 1) * CH)
                x_ch = xa.tile([128, 16, CH], DT, name=f"x_{n}", tag="x")
                xd = xT[:, sl].rearrange("(k p) t -> p k t", p=128)
                for kg in range(4):
                    nc.sync.dma_start(out=x_ch[:, 4 * kg:4 * kg + 4, :],
                                      in_=xd[:, 4 * kg:4 * kg + 4, :])
                x_tiles[n] = x_ch

            dma_x(0)
            dma_w(wq_sb, wqT)
            nc.sync.dma_start(out=cos_sb, in_=cos2)
            nc.sync.dma_start(out=sin_sb, in_=sin2s)

            for n in range(NCH):
                sl = slice(n * CH, (n + 1) * CH)
                if n not in x_tiles:
                    dma_x(n)
                x_ch = x_tiles.pop(n)
                # --- Q/K + RoPE ---
                for w_sb, dst, wn in ((wq_sb, qT_sb, "q"), (wk_sb, kT_sb, "k")):
                    for m in range(NPR):
                        ps = ps1.tile([128, CH], fp32,
                                      name=f"ps{wn}{m}_{n}", tag="ps1")
                        for k in range(16):
                            nc.tensor.matmul(
                                ps,
                                lhsT=mm(w_sb[:, k, m * 128:(m + 1) * 128]),
                                rhs=mm(x_ch[:, k, :]),
                                start=(k == 0), stop=(k == 15))
                        # rot = partition-shift of ps via 4 ACT copies
                        # (PSUM source, so the SBUF same-start rule only
                        # sees the output), then 2 full-width DVE muls +
                        # GPS add.  Spreads rope across ACT/DVE/GPS.
                        rot = rt.tile([128, CH], fp32, name=f"ro{wn}{m}_{n}", tag="ro")
                        for (d, s_) in ((0, 32), (32, 0), (64, 96), (96, 64)):
                            nc.scalar.copy(rot[d:d + 32, :], ps[s_:s_ + 32, :])
                        t1 = rt.tile([128, CH], fp32, name=f"t1{wn}{m}_{n}", tag="t1")
                        t2 = rt.tile([128, CH], fp32, name=f"t2{wn}{m}_{n}", tag="t2")
                        nc.vector.tensor_mul(t1, ps, cos_sb[:, sl])
                        nc.vector.tensor_mul(t2, rot, sin_sb[:, sl])
                        nc.gpsimd.tensor_add(dst[:, m, sl], t1, t2)
                    if n == 0 and wn == "q":
                        dma_w(wk_sb, wkT)      # after Q's prefetch drained
                if n == 0:
                    dma_w(wv_sb, wvT)
                    dma_x(1)
                # --- V (same x chunk) ---
                for s_ in range(CH // 128):
                    t16 = (n * CH) // 128 + s_
                    ps = ps1.tile([128, BLK], fp32, name=f"psv{t16}", tag="ps1")
                    for k in range(16):
                        nc.tensor.matmul(
                            ps,
                            lhsT=mm(x_ch[:, k, s_ * 128:(s_ + 1) * 128]),
                            rhs=mm(wv_sb[:, k, :]),
                            start=(k == 0), stop=(k == 15))
                    dstv = v_sb[:, t16, :].rearrange("p (h c) -> p h c", c=65)[:, :, 0:64]
                    nc.vector.tensor_copy(dstv, ps.rearrange("p (h c) -> p h c", c=64))

    with tc_ctx as tc:
        with tc.tile_pool(name="qk_sb", bufs=1) as qk_pool, \
             tc.tile_pool(name="v_sb_pool", bufs=1) as v_pool:
            qT_sb = qk_pool.tile([128, NPR, S], SDT)
            kT_sb = qk_pool.tile([128, NPR, S], SDT)
            v_sb = v_pool.tile([128, NT, HEADS_PER_CORE * 65], SDT)
            ones_ap = v_sb.rearrange("p t (h c) -> p t h c", c=65)[:, :, :, 64:65]
            nc.vector.memset(ones_ap, 1.0)

            phase1(tc, qT_sb, kT_sb, v_sb)

            # -------- phase 2+3: attention + fused out-projection ----
            with tc.tile_pool(name="attn_pool", bufs=1) as apool, \
                 tc.tile_pool(name="wo", bufs=1) as wop:
                attn_sb = apool.tile([128, NPR, S], SDT)
                wo_sb = wop.tile([128, NPR, HID], DT)
                nc.sync.dma_start(out=wo_sb,
                                  in_=woT.rearrange("(r p) o -> p r o", p=128))
                with tc.tile_pool(name="ppool", bufs=3) as ppool, \
                     tc.tile_pool(name="npool", bufs=2) as npool, \
                     tc.tile_pool(name="mpool", bufs=2) as mpool, \
                     tc.tile_pool(name="ostage", bufs=2) as ostage, \
                     tc.tile_pool(name="pss", bufs=2, space="PSUM") as pss, \
                     tc.tile_pool(name="psav", bufs=1, space="PSUM") as psav, \
                     tc.tile_pool(name="opool", bufs=2, space="PSUM") as opool:

                    from collections import deque
                    pending_out = deque()

                    def emit_outproj_group(t16, no):
                        o_ps = opool.tile([128, 512], fp32,
                                          name=f"o{t16}_{no}", tag="o")
                        for pr_ in range(NPR):
                            nc.tensor.matmul(
                                o_ps,
                                lhsT=mm(attn_sb[:, pr_,
                                                t16 * 128:(t16 + 1) * 128]),
                                rhs=mm(wo_sb[:, pr_,
                                             no * 512:(no + 1) * 512]),
                                start=(pr_ == 0), stop=(pr_ == NPR - 1))
                        o_sb = ostage.tile([128, 512], fp32,
                                           name=f"os{t16}_{no}", tag="os")
                        nc.vector.tensor_copy(o_sb, o_ps)
                        nc.sync.dma_start(
                            out=out[t16 * 128:(t16 + 1) * 128,
                                    no * 512:(no + 1) * 512],
                            in_=o_sb)

                    def drain_one():
                        if pending_out:
                            t16, no = pending_out.popleft()
                            emit_outproj_group(t16, no)

                    def attend(pr, j, mask_col):
                        n_i = 4 * j + 4 if causal else NT
                        # full-bank AV tiles: rows 0-64 accumulate
                        # (out.T | denom); rows 64-127 later hold the
                        # PE-broadcast reciprocal
                        av = [psav.tile([128, 512], fp32,
                                        name=f"av{half}_{pr}_{j}", tag=f"av{half}")
                              for half in range(2)]
                        s_t, p_t, nw_t = {}, {}, {}

                        def emit_S(i):
                            # causal boundary tiles only need the tq
                            # suffix [512j+off, 512(j+1))
                            off = max(0, 128 * (i - 4 * j)) if causal else 0
                            nw = 512 - off
                            nw_t[i] = (off, nw)
                            s_ps = pss.tile([128, 1024], fp32,
                                            name=f"s_{pr}_{j}_{i}", tag="s")
                            for half in range(2):
                                r0 = 64 * half
                                nc.tensor.matmul(
                                    s_ps[:, half * 512:half * 512 + nw],
                                    lhsT=mm(kT_sb[r0:r0 + 64, pr,
                                                  i * 128:(i + 1) * 128]),
                                    rhs=mm(qT_sb[r0:r0 + 64, pr,
                                                 j * 512 + off:(j + 1) * 512]),
                                    start=True, stop=True,
                                    tile_position=(r0, 0))
                            s_t[i] = s_ps

                        def emit_exp(i):
                            off, nw = nw_t[i]
                            s_ps = s_t[i]
                            s_v = s_ps.rearrange("q (h t) -> q h t", h=2)[:, :, 0:nw]
                            p = ppool.tile([128, 1024], SDT,
                                           name=f"p_{pr}_{j}_{i}", tag="p")
                            p_v = p.rearrange("q (h t) -> q h t", h=2)[:, :, 0:nw]
                            if causal:
                                nc.scalar.activation(p_v, s_v,
                                                     mybir.ActivationFunctionType.Exp,
                                                     scale=0.125)
                                if i >= 4 * j:
                                    # keep iff tq - tk >= 0 (base is 0 on
                                    # boundary tiles thanks to the suffix)
                                    nc.gpsimd.affine_select(
                                        out=p_v, in_=p_v,
                                        compare_op=mybir.AluOpType.is_ge,
                                        fill=0.0,
                                        base=512 * j + off - 128 * i,
                                        pattern=[[0, 2], [1, nw]],
                                        channel_multiplier=-1)
                            else:
                                tmp = ppool.tile([128, 1024], fp32,
                                                 name=f"pt_{pr}_{j}_{i}", tag="pt")
                                for half in range(2):
                                    nc.vector.scalar_tensor_tensor(
                                        out=tmp[:, half * 512:(half + 1) * 512],
                                        in0=s_ps[:, half * 512:(half + 1) * 512],
                                        scalar=0.125,
                                        in1=mask_col[:, i, :],
                                        op0=mybir.AluOpType.mult,
                                        op1=mybir.AluOpType.add)
                                nc.scalar.activation(p, tmp,
                                                     mybir.ActivationFunctionType.Exp)
                            p_t[i] = p

                        def emit_AV(i, first, last):
                            off, nw = nw_t[i]
                            p = p_t[i]
                            for half in range(2):
                                h = 2 * pr + half
                                nc.tensor.matmul(
                                    av[half][0:65, off:512],
                                    lhsT=mm(v_sb[:, i, 65 * h:65 * h + 65]),
                                    rhs=mm(p[:, half * 512:half * 512 + nw]),
                                    start=first, stop=last)

                        # software pipeline: S runs 2 tiles ahead of AV,
                        # with outproj matmul groups drained between
                        # iterations as PE filler while ACT runs exp.
                        # (ascending i is required: AV i=0 writes the full
                        # 512 with start=True, clearing has_written before
                        # the suffix-trimmed boundary tiles accumulate)
                        order = list(range(n_i))
                        first_i, last_i = order[0], order[-1]
                        emit_S(order[0])
                        if n_i > 1:
                            emit_S(order[1])
                        emit_exp(order[0])
                        for ii, i in enumerate(order):
                            if ii + 2 < n_i:
                                emit_S(order[ii + 2])
                            if ii + 1 < n_i:
                                emit_exp(order[ii + 1])
                            emit_AV(i, i == first_i, i == last_i)
                            drain_one()

                        # normalize rows 0..63 by row 64: fast approximate
                        # reciprocal on DVE (~51 ULP, plenty for a softmax
                        # denominator), partition-broadcast on GPSIMD, then
                        # one DVE multiply into attn_sb.  Keeps the PE
                        # stream free of ops that wait on DVE round-trips.
                        for half in range(2):
                            r0 = 64 * half
                            den = npool.tile([1, 512], fp32,
                                             name=f"den{half}_{pr}_{j}",
                                             tag=f"den{half}")
                            # approx recip is a bitwise custom-DVE op and
                            # must read SBUF, not PSUM — bounce the row.
                            nc.vector.tensor_copy(den, av[half][64:65, :])
                            rec = npool.tile([1, 512], fp32,
                                             name=f"rec{half}_{pr}_{j}",
                                             tag=f"rec{half}")
                            nc.vector.reciprocal_approx_fast(rec, den)
                            bcr = npool.tile([64, 512], fp32,
                                             name=f"bc{half}_{pr}_{j}",
                                             tag=f"bc{half}")
                            nc.gpsimd.partition_broadcast(bcr, rec)
                            nc.vector.tensor_mul(
                                attn_sb[r0:r0 + 64, pr, j * 512:(j + 1) * 512],
                                av[half][0:64, :], bcr)

                    for j in range(4):
                        if causal:
                            mask_col = None
                        else:
                            mask_col = mpool.tile([128, NT, 512],
                                                  mybir.dt.bfloat16,
                                                  name=f"mc{j}", tag="mc")
                            nc.sync.dma_start(
                                out=mask_col,
                                in_=maskT[:, j * 512:(j + 1) * 512]
                                .rearrange("(i p) t -> p i t", p=128))
                        for pr in range(NPR):
                            attend(pr, j, mask_col)
                        # column j's attention rows are complete; queue
                        # its out-projection as PE filler for column j+1
                        for t16 in range(4 * j, 4 * j + 4):
                            for no in range(4):
                                pending_out.append((t16, no))
                    while pending_out:
                        drain_one()
    nc.compile()
    return nc


def _get_program(variant):
    if variant not in _PROGRAMS:
        _PROGRAMS[variant] = build_program(variant)
    return _PROGRAMS[variant]


def make_in_maps(hidden_states, attention_mask, position_ids, Wq, Wk, Wv, Wo,
                 variant):
    npdt = ml_dtypes.bfloat16
    cos, sin = _rope_cache()
    in_maps = []
    for c in range(N_CORES):
        b = c // (N_CORES // 2)
        hb = c % (N_CORES // 2)
        rs = slice(hb * BLK, (hb + 1) * BLK)
        pos = np.asarray(position_ids[b]).astype(np.int64)
        cos_b = cos[pos].T.astype(np.float32)     # [64, S]
        sin_b = sin[pos].T.astype(np.float32)
        sin_s = np.concatenate([-sin_b[:32], sin_b[32:]], axis=0)
        m = {
            "xT": np.ascontiguousarray(np.asarray(hidden_states)[b].T).astype(npdt),
            "wqT": np.ascontiguousarray(np.asarray(Wq)[rs].T).astype(npdt),
            "wkT": np.ascontiguousarray(np.asarray(Wk)[rs].T).astype(npdt),
            "wvT": np.ascontiguousarray(np.asarray(Wv)[rs].T).astype(npdt),
            "woT": np.ascontiguousarray(np.asarray(Wo)[:, rs].T).astype(npdt),
            "cos2": np.ascontiguousarray(
                np.concatenate([cos_b, cos_b], axis=0)).astype(npdt),
            "sin2s": np.ascontiguousarray(
                np.concatenate([sin_s, sin_s], axis=0)).astype(npdt),
        }
        if variant == "general":
            m["maskT"] = np.ascontiguousarray(
                np.asarray(attention_mask)[b, 0].T).astype(ml_dtypes.bfloat16)
        in_maps.append(m)
    return in_maps


def detect_causal(attention_mask):
    am = np.asarray(attention_mask)
    if am.shape != (2, 1, S, S):
        return False
    neg = np.float32(np.finfo(np.float32).min)
    canonical = np.where(np.tril(np.ones((S, S), dtype=bool)), np.float32(0.0), neg)
    return bool(np.array_equal(am[0, 0], canonical) and
                np.array_equal(am[1, 0], canonical))


def kernel(hidden_states, attention_mask, position_ids, Wq, Wk, Wv, Wo):
    hidden_states = np.asarray(hidden_states, dtype=np.float32)
    attention_mask = np.asarray(attention_mask, dtype=np.float32)
    Wq, Wk, Wv, Wo = (np.asarray(w, dtype=np.float32) for w in (Wq, Wk, Wv, Wo))

    variant = "causal" if detect_causal(attention_mask) else "general"
    nc = _get_program(variant)
    in_maps = make_in_maps(hidden_states, attention_mask, position_ids,
                           Wq, Wk, Wv, Wo, variant)

    from concourse import bass2jax
    results = bass2jax.run_bass_via_pjrt(nc, in_maps, n_cores=N_CORES)

    out = np.zeros((2, S, HID), dtype=np.float64)
    for c in range(N_CORES):
        b = c // (N_CORES // 2)
        out[b] += results[c]["out"].astype(np.float64)
    return out.astype(np.float32)


# revision 15
# speedup vs baseline: 1.1853x; 1.0105x over previous
"""MiniCPM attention block on 8 Trainium2 NeuronCores.

Sharding: core c handles batch b = c // 4 and the 8 heads
[ (c%4)*8, (c%4)*8 + 8 ) of that batch (tensor-parallel over heads +
data-parallel over batch).  Each core computes a partial output
x @ block-of-Wo.T of shape [S, HID]; the host sums the 4 partials per
batch.  No collectives.

Device pipeline per core (S=2048 tokens, 8 heads of d=64):
  1. per 512-token chunk: qT/kT = (x @ Wq_s.T).T with RoPE fused into
     the PSUM->SBUF evict, and v = x @ Wv_s.T (stored [tk, 8*65] with a
     ones column per head) — one x load per chunk shared by Q/K/V, all
     12 PSUM groups rotating through one 8-slot pool.
  2. per head-pair, per tq-chunk j (512): S.T tiles [tk 128, tq 512]
     on PE (2 heads packed via row tiling -> concurrent), exp on ACT
     (pair-fused into one [128,1024] op), causal zeroing via gpsimd
     affine_select, AV accumulation on PE with the ones column giving
     the softmax denominator for free.  Normalize via DVE reciprocal +
     a K=1 PE broadcast matmul (ones^T @ rec -> psum rows 64:128) +
     DVE copy/mul — no DRAM bounce.  The i-loop is software-pipelined
     (S two tiles ahead of AV) so PE never waits on the ACT exp
     round-trip.  Causal boundary tiles compute only the valid tq
     suffix.
  3. out_partial = attn_outT.T @ Wo_s.T in [128,512] PSUM groups with
     DVE evicts, interleaved per tq-column so PE has dense work while
     ACT runs the next column's exps.

Self-contained: shapes hardcoded from the problem spec.
"""
import numpy as np
import ml_dtypes

S = 2048
HID = 2048
NH = 32
DH = 64
N_CORES = 8
HEADS_PER_CORE = NH // (N_CORES // 2)   # 8
BLK = HEADS_PER_CORE * DH               # 512
ROPE_BASE = 10000.0

_PROGRAMS = {}


def _rope_cache():
    inv_freq = 1.0 / (ROPE_BASE ** (np.arange(0, DH, 2, dtype=np.float32) / DH))
    t = np.arange(S, dtype=np.float32)
    freqs = np.outer(t, inv_freq)                     # [S, 32]
    emb = np.concatenate([freqs, freqs], axis=-1)     # [S, 64]
    return np.cos(emb), np.sin(emb)


def build_program(variant="causal"):
    """Build the Bacc program (one NEFF, run SPMD on 8 cores)."""
    import concourse.bass as bass
    import concourse.mybir as mybir
    import concourse.tile as tile
    from concourse import bacc

    fp32 = mybir.dt.float32
    f32r = mybir.dt.float32r
    DT = mybir.dt.bfloat16      # dram/lhs/rhs matmul dtype
    SDT = mybir.dt.bfloat16     # on-chip storage for q/k/v/p/attn
    CH = 512                    # token chunk for phase 1

    def mm(ap):
        return ap.bitcast(DT) if DT is not ap.dtype else ap

    causal = variant == "causal"
    NCH = S // CH            # phase-1 token chunks
    NT = S // 128            # 16 token tiles
    NPR = 4                  # head pairs

    nc = bacc.Bacc("TRN2", target_bir_lowering=False, debug=False,
                   enable_asserts=False, num_devices=N_CORES)

    xT = nc.dram_tensor("xT", [HID, S], DT, kind="ExternalInput").ap()
    wqT = nc.dram_tensor("wqT", [HID, BLK], DT, kind="ExternalInput").ap()
    wkT = nc.dram_tensor("wkT", [HID, BLK], DT, kind="ExternalInput").ap()
    wvT = nc.dram_tensor("wvT", [HID, BLK], DT, kind="ExternalInput").ap()
    woT = nc.dram_tensor("woT", [BLK, HID], DT, kind="ExternalInput").ap()
    cos2 = nc.dram_tensor("cos2", [128, S], DT, kind="ExternalInput").ap()
    sin2s = nc.dram_tensor("sin2s", [128, S], DT, kind="ExternalInput").ap()
    if not causal:
        maskT = nc.dram_tensor("maskT", [S, S], mybir.dt.bfloat16,
                               kind="ExternalInput").ap()
    out = nc.dram_tensor("out", [S, HID], fp32, kind="ExternalOutput").ap()

    tc_ctx = tile.TileContext(nc)

    def phase1(tc, qT_sb, kT_sb, v_sb):
        with tc.tile_pool(name="consts", bufs=1) as cpool, \
             tc.tile_pool(name="wall", bufs=1) as wpool, \
             tc.tile_pool(name="xa", bufs=2) as xa, \
             tc.tile_pool(name="ropetmp", bufs=2) as rt, \
             tc.tile_pool(name="ps1", bufs=8, space="PSUM") as ps1:
            # DMA emission order matters for startup latency: the first
            # matmul group needs x chunk 0 + wq only; cos/sin only matter
            # ~a dozen matmuls later; wk/wv are emitted mid-chunk-0 so
            # they don't steal HBM bandwidth from the critical prefetch.
            wq_sb = wpool.tile([128, 16, BLK], DT)
            wk_sb = wpool.tile([128, 16, BLK], DT)
            wv_sb = wpool.tile([128, 16, BLK], DT)
            cos_sb = cpool.tile([128, S], DT)
            sin_sb = cpool.tile([128, S], DT)

            def dma_w(w_sb, w_d):
                wd = w_d.rearrange("(k p) m -> p k m", p=128)
                for kg in range(4):
                    nc.sync.dma_start(out=w_sb[:, 4 * kg:4 * kg + 4, :],
                                      in_=wd[:, 4 * kg:4 * kg + 4, :])

            x_tiles = {}

            def dma_x(n):
                sl = slice(n * CH, (n + 1) * CH)
                x_ch = xa.tile([128, 16, CH], DT, name=f"x_{n}", tag="x")
                xd = xT[:, sl].rearrange("(k p) t -> p k t", p=128)
                for kg in range(4):
                    nc.sync.dma_start(out=x_ch[:, 4 * kg:4 * kg + 4, :],
                                      in_=xd[:, 4 * kg:4 * kg + 4, :])
                x_tiles[n] = x_ch

            dma_x(0)
            dma_w(wq_sb, wqT)
            nc.sync.dma_start(out=cos_sb, in_=cos2)
            nc.sync.dma_start(out=sin_sb, in_=sin2s)

            for n in range(NCH):
                sl = slice(n * CH, (n + 1) * CH)
                if n not in x_tiles:
                    dma_x(n)
                x_ch = x_tiles.pop(n)
                # --- Q/K + RoPE ---
                for w_sb, dst, wn in ((wq_sb, qT_sb, "q"), (wk_sb, kT_sb, "k")):
                    for m in range(NPR):
                        ps = ps1.tile([128, CH], fp32,
                                      name=f"ps{wn}{m}_{n}", tag="ps1")
                        for k in range(16):
                            nc.tensor.matmul(
                                ps,
                                lhsT=mm(w_sb[:, k, m * 128:(m + 1) * 128]),
                                rhs=mm(x_ch[:, k, :]),
                                start=(k == 0), stop=(k == 15))
                        # rot = partition-shift of ps via 4 ACT copies
                        # (PSUM source, so the SBUF same-start rule only
                        # sees the output), then 2 full-width DVE muls +
                        # GPS add.  Spreads rope across ACT/DVE/GPS.
                        rot = rt.tile([128, CH], fp32, name=f"ro{wn}{m}_{n}", tag="ro")
                        # last chunk's shifts go on DVE so the phase-2 exps
                        # don't queue behind an ACT backlog at the boundary
                        shift_eng = nc.vector if n == NCH - 1 else nc.scalar
                        for (d, s_) in ((0, 32), (32, 0), (64, 96), (96, 64)):
                            if shift_eng is nc.scalar:
                                nc.scalar.copy(rot[d:d + 32, :], ps[s_:s_ + 32, :])
                            else:
                                nc.vector.tensor_copy(rot[d:d + 32, :],
                                                      ps[s_:s_ + 32, :])
                        t1 = rt.tile([128, CH], fp32, name=f"t1{wn}{m}_{n}", tag="t1")
                        t2 = rt.tile([128, CH], fp32, name=f"t2{wn}{m}_{n}", tag="t2")
                        nc.vector.tensor_mul(t1, ps, cos_sb[:, sl])
                        nc.vector.tensor_mul(t2, rot, sin_sb[:, sl])
                        nc.gpsimd.tensor_add(dst[:, m, sl], t1, t2)
                    if n == 0 and wn == "q":
                        dma_w(wk_sb, wkT)      # after Q's prefetch drained
                if n == 0:
                    dma_w(wv_sb, wvT)
                    dma_x(1)
                # --- V (same x chunk) ---
                for s_ in range(CH // 128):
                    t16 = (n * CH) // 128 + s_
                    ps = ps1.tile([128, BLK], fp32, name=f"psv{t16}", tag="ps1")
                    for k in range(16):
                        nc.tensor.matmul(
                            ps,
                            lhsT=mm(x_ch[:, k, s_ * 128:(s_ + 1) * 128]),
                            rhs=mm(wv_sb[:, k, :]),
                            start=(k == 0), stop=(k == 15))
                    dstv = v_sb[:, t16, :].rearrange("p (h c) -> p h c", c=65)[:, :, 0:64]
                    nc.vector.tensor_copy(dstv, ps.rearrange("p (h c) -> p h c", c=64))

    with tc_ctx as tc:
        with tc.tile_pool(name="qk_sb", bufs=1) as qk_pool, \
             tc.tile_pool(name="v_sb_pool", bufs=1) as v_pool:
            qT_sb = qk_pool.tile([128, NPR, S], SDT)
            kT_sb = qk_pool.tile([128, NPR, S], SDT)
            v_sb = v_pool.tile([128, NT, HEADS_PER_CORE * 65], SDT)
            ones_ap = v_sb.rearrange("p t (h c) -> p t h c", c=65)[:, :, :, 64:65]
            nc.vector.memset(ones_ap, 1.0)

            phase1(tc, qT_sb, kT_sb, v_sb)

            # -------- phase 2+3: attention + fused out-projection ----
            with tc.tile_pool(name="attn_pool", bufs=1) as apool, \
                 tc.tile_pool(name="wo", bufs=1) as wop:
                attn_sb = apool.tile([128, NPR, S], SDT)
                wo_sb = wop.tile([128, NPR, HID], DT)
                nc.sync.dma_start(out=wo_sb,
                                  in_=woT.rearrange("(r p) o -> p r o", p=128))
                with tc.tile_pool(name="ppool", bufs=3) as ppool, \
                     tc.tile_pool(name="npool", bufs=2) as npool, \
                     tc.tile_pool(name="mpool", bufs=2) as mpool, \
                     tc.tile_pool(name="ostage", bufs=2) as ostage, \
                     tc.tile_pool(name="pss", bufs=2, space="PSUM") as pss, \
                     tc.tile_pool(name="psav", bufs=1, space="PSUM") as psav, \
                     tc.tile_pool(name="opool", bufs=2, space="PSUM") as opool:

                    from collections import deque
                    pending_out = deque()

                    def emit_outproj_group(t16, no):
                        o_ps = opool.tile([128, 512], fp32,
                                          name=f"o{t16}_{no}", tag="o")
                        for pr_ in range(NPR):
                            nc.tensor.matmul(
                                o_ps,
                                lhsT=mm(attn_sb[:, pr_,
                                                t16 * 128:(t16 + 1) * 128]),
                                rhs=mm(wo_sb[:, pr_,
                                             no * 512:(no + 1) * 512]),
                                start=(pr_ == 0), stop=(pr_ == NPR - 1))
                        o_sb = ostage.tile([128, 512], fp32,
                                           name=f"os{t16}_{no}", tag="os")
                        nc.vector.tensor_copy(o_sb, o_ps)
                        nc.sync.dma_start(
                            out=out[t16 * 128:(t16 + 1) * 128,
                                    no * 512:(no + 1) * 512],
                            in_=o_sb)

                    def drain_one():
                        if pending_out:
                            t16, no = pending_out.popleft()
                            emit_outproj_group(t16, no)

                    def attend(pr, j, mask_col):
                        n_i = 4 * j + 4 if causal else NT
                        # full-bank AV tiles: rows 0-64 accumulate
                        # (out.T | denom); rows 64-127 later hold the
                        # PE-broadcast reciprocal
                        av = [psav.tile([128, 512], fp32,
                                        name=f"av{half}_{pr}_{j}", tag=f"av{half}")
                              for half in range(2)]
                        s_t, p_t, nw_t = {}, {}, {}

                        def emit_S(i):
                            # causal boundary tiles only need the tq
                            # suffix [512j+off, 512(j+1))
                            off = max(0, 128 * (i - 4 * j)) if causal else 0
                            nw = 512 - off
                            nw_t[i] = (off, nw)
                            s_ps = pss.tile([128, 1024], fp32,
                                            name=f"s_{pr}_{j}_{i}", tag="s")
                            for half in range(2):
                                r0 = 64 * half
                                nc.tensor.matmul(
                                    s_ps[:, half * 512:half * 512 + nw],
                                    lhsT=mm(kT_sb[r0:r0 + 64, pr,
                                                  i * 128:(i + 1) * 128]),
                                    rhs=mm(qT_sb[r0:r0 + 64, pr,
                                                 j * 512 + off:(j + 1) * 512]),
                                    start=True, stop=True,
                                    tile_position=(r0, 0))
                            s_t[i] = s_ps

                        def emit_exp(i):
                            off, nw = nw_t[i]
                            s_ps = s_t[i]
                            s_v = s_ps.rearrange("q (h t) -> q h t", h=2)[:, :, 0:nw]
                            p = ppool.tile([128, 1024], SDT,
                                           name=f"p_{pr}_{j}_{i}", tag="p")
                            p_v = p.rearrange("q (h t) -> q h t", h=2)[:, :, 0:nw]
                            if causal:
                                nc.scalar.activation(p_v, s_v,
                                                     mybir.ActivationFunctionType.Exp,
                                                     scale=0.125)
                                if i >= 4 * j:
                                    # keep iff tq - tk >= 0 (base is 0 on
                                    # boundary tiles thanks to the suffix)
                                    nc.gpsimd.affine_select(
                                        out=p_v, in_=p_v,
                                        compare_op=mybir.AluOpType.is_ge,
                                        fill=0.0,
                                        base=512 * j + off - 128 * i,
                                        pattern=[[0, 2], [1, nw]],
                                        channel_multiplier=-1)
                            else:
                                tmp = ppool.tile([128, 1024], fp32,
                                                 name=f"pt_{pr}_{j}_{i}", tag="pt")
                                for half in range(2):
                                    nc.vector.scalar_tensor_tensor(
                                        out=tmp[:, half * 512:(half + 1) * 512],
                                        in0=s_ps[:, half * 512:(half + 1) * 512],
                                        scalar=0.125,
                                        in1=mask_col[:, i, :],
                                        op0=mybir.AluOpType.mult,
                                        op1=mybir.AluOpType.add)
                                nc.scalar.activation(p, tmp,
                                                     mybir.ActivationFunctionType.Exp)
                            p_t[i] = p

                        def emit_AV(i, first, last):
                            off, nw = nw_t[i]
                            p = p_t[i]
                            for half in range(2):
                                h = 2 * pr + half
                                nc.tensor.matmul(
                                    av[half][0:65, off:512],
                                    lhsT=mm(v_sb[:, i, 65 * h:65 * h + 65]),
                                    rhs=mm(p[:, half * 512:half * 512 + nw]),
                                    start=first, stop=last)

                        # software pipeline: S runs 2 tiles ahead of AV,
                        # with outproj matmul groups drained between
                        # iterations as PE filler while ACT runs exp.
                        # (ascending i is required: AV i=0 writes the full
                        # 512 with start=True, clearing has_written before
                        # the suffix-trimmed boundary tiles accumulate)
                        order = list(range(n_i))
                        first_i, last_i = order[0], order[-1]
                        emit_S(order[0])
                        if n_i > 1:
                            emit_S(order[1])
                        emit_exp(order[0])
                        for ii, i in enumerate(order):
                            if ii + 2 < n_i:
                                emit_S(order[ii + 2])
                            if ii + 1 < n_i:
                                emit_exp(order[ii + 1])
                            # drain BEFORE AV: the outproj group gives PE
                            # ready work to cover the exp->AV semaphore
                            # latency (PE is FIFO; AV would head-of-line
                            # block an already-ready outproj group)
                            drain_one()
                            emit_AV(i, i == first_i, i == last_i)

                        # normalize rows 0..63 by row 64: fast approximate
                        # reciprocal on DVE (~51 ULP, plenty for a softmax
                        # denominator), partition-broadcast on GPSIMD, then
                        # one DVE multiply into attn_sb.  Keeps the PE
                        # stream free of ops that wait on DVE round-trips.
                        for half in range(2):
                            r0 = 64 * half
                            den = npool.tile([1, 512], fp32,
                                             name=f"den{half}_{pr}_{j}",
                                             tag=f"den{half}")
                            # approx recip is a bitwise custom-DVE op and
                            # must read SBUF, not PSUM — bounce the row.
                            nc.vector.tensor_copy(den, av[half][64:65, :])
                            rec = npool.tile([1, 512], fp32,
                                             name=f"rec{half}_{pr}_{j}",
                                             tag=f"rec{half}")
                            nc.vector.reciprocal_approx_fast(rec, den)
                            bcr = npool.tile([64, 512], fp32,
                                             name=f"bc{half}_{pr}_{j}",
                                             tag=f"bc{half}")
                            nc.gpsimd.partition_broadcast(bcr, rec)
                            nc.vector.tensor_mul(
                                attn_sb[r0:r0 + 64, pr, j * 512:(j + 1) * 512],
                                av[half][0:64, :], bcr)

                    for j in range(4):
                        if causal:
                            mask_col = None
                        else:
                            mask_col = mpool.tile([128, NT, 512],
                                                  mybir.dt.bfloat16,
                                                  name=f"mc{j}", tag="mc")
                            nc.sync.dma_start(
                                out=mask_col,
                                in_=maskT[:, j * 512:(j + 1) * 512]
                                .rearrange("(i p) t -> p i t", p=128))
                        for pr in range(NPR):
                            attend(pr, j, mask_col)
                        # column j's attention rows are complete; queue
                        # its out-projection as PE filler for column j+1
                        for t16 in range(4 * j, 4 * j + 4):
                            for no in range(4):
                                pending_out.append((t16, no))
                    while pending_out:
                        drain_one()
    nc.compile()
    return nc


def _get_program(variant):
    if variant not in _PROGRAMS:
        _PROGRAMS[variant] = build_program(variant)
    return _PROGRAMS[variant]


def make_in_maps(hidden_states, attention_mask, position_ids, Wq, Wk, Wv, Wo,
                 variant):
    npdt = ml_dtypes.bfloat16
    cos, sin = _rope_cache()
    in_maps = []
    for c in range(N_CORES):
        b = c // (N_CORES // 2)
        hb = c % (N_CORES // 2)
        rs = slice(hb * BLK, (hb + 1) * BLK)
        pos = np.asarray(position_ids[b]).astype(np.int64)
        cos_b = cos[pos].T.astype(np.float32)     # [64, S]
        sin_b = sin[pos].T.astype(np.float32)
        sin_s = np.concatenate([-sin_b[:32], sin_b[32:]], axis=0)
        m = {
            "xT": np.ascontiguousarray(np.asarray(hidden_states)[b].T).astype(npdt),
            "wqT": np.ascontiguousarray(np.asarray(Wq)[rs].T).astype(npdt),
            "wkT": np.ascontiguousarray(np.asarray(Wk)[rs].T).astype(npdt),
            "wvT": np.ascontiguousarray(np.asarray(Wv)[rs].T).astype(npdt),
            "woT": np.ascontiguousarray(np.asarray(Wo)[:, rs].T).astype(npdt),
            "cos2": np.ascontiguousarray(
                np.concatenate([cos_b, cos_b], axis=0)).astype(npdt),
            "sin2s": np.ascontiguousarray(
                np.concatenate([sin_s, sin_s], axis=0)).astype(npdt),
        }
        if variant == "general":
            m["maskT"] = np.ascontiguousarray(
                np.asarray(attention_mask)[b, 0].T).astype(ml_dtypes.bfloat16)
        in_maps.append(m)
    return in_maps


def detect_causal(attention_mask):
    am = np.asarray(attention_mask)
    if am.shape != (2, 1, S, S):
        return False
    neg = np.float32(np.finfo(np.float32).min)
    canonical = np.where(np.tril(np.ones((S, S), dtype=bool)), np.float32(0.0), neg)
    return bool(np.array_equal(am[0, 0], canonical) and
                np.array_equal(am[1, 0], canonical))


def kernel(hidden_states, attention_mask, position_ids, Wq, Wk, Wv, Wo):
    hidden_states = np.asarray(hidden_states, dtype=np.float32)
    attention_mask = np.asarray(attention_mask, dtype=np.float32)
    Wq, Wk, Wv, Wo = (np.asarray(w, dtype=np.float32) for w in (Wq, Wk, Wv, Wo))

    variant = "causal" if detect_causal(attention_mask) else "general"
    nc = _get_program(variant)
    in_maps = make_in_maps(hidden_states, attention_mask, position_ids,
                           Wq, Wk, Wv, Wo, variant)

    from concourse import bass2jax
    results = bass2jax.run_bass_via_pjrt(nc, in_maps, n_cores=N_CORES)

    out = np.zeros((2, S, HID), dtype=np.float64)
    for c in range(N_CORES):
        b = c // (N_CORES // 2)
        out[b] += results[c]["out"].astype(np.float64)
    return out.astype(np.float32)


# revision 19
# speedup vs baseline: 1.2705x; 1.0719x over previous
"""MiniCPM attention block on 8 Trainium2 NeuronCores.

Sharding: core c handles batch b = c // 4 and the 8 heads
[ (c%4)*8, (c%4)*8 + 8 ) of that batch (tensor-parallel over heads +
data-parallel over batch).  Each core computes a partial output
x @ block-of-Wo.T of shape [S, HID]; the host sums the 4 partials per
batch.  No collectives.

Device pipeline per core (S=2048 tokens, 8 heads of d=64):
  1. per 512-token chunk: qT/kT = (x @ Wq_s.T).T with RoPE fused into
     the PSUM->SBUF evict, and v = x @ Wv_s.T (stored [tk, 8*65] with a
     ones column per head) — one x load per chunk shared by Q/K/V, all
     12 PSUM groups rotating through one 8-slot pool.
  2. per head-pair, per tq-chunk j (512): S.T tiles [tk 128, tq 512]
     on PE (2 heads packed via row tiling -> concurrent), exp on ACT
     (pair-fused into one [128,1024] op), causal zeroing via gpsimd
     affine_select, AV accumulation on PE with the ones column giving
     the softmax denominator for free.  Normalize via DVE reciprocal +
     a K=1 PE broadcast matmul (ones^T @ rec -> psum rows 64:128) +
     DVE copy/mul — no DRAM bounce.  The i-loop is software-pipelined
     (S two tiles ahead of AV) so PE never waits on the ACT exp
     round-trip.  Causal boundary tiles compute only the valid tq
     suffix.
  3. out_partial = attn_outT.T @ Wo_s.T in [128,512] PSUM groups with
     DVE evicts, interleaved per tq-column so PE has dense work while
     ACT runs the next column's exps.

Self-contained: shapes hardcoded from the problem spec.
"""
import numpy as np
import ml_dtypes

S = 2048
HID = 2048
NH = 32
DH = 64
N_CORES = 8
HEADS_PER_CORE = NH // (N_CORES // 2)   # 8
BLK = HEADS_PER_CORE * DH               # 512
ROPE_BASE = 10000.0

_PROGRAMS = {}


def _rope_cache():
    inv_freq = 1.0 / (ROPE_BASE ** (np.arange(0, DH, 2, dtype=np.float32) / DH))
    t = np.arange(S, dtype=np.float32)
    freqs = np.outer(t, inv_freq)                     # [S, 32]
    emb = np.concatenate([freqs, freqs], axis=-1)     # [S, 64]
    return np.cos(emb), np.sin(emb)


def build_program(variant="causal"):
    """Build the Bacc program (one NEFF, run SPMD on 8 cores)."""
    import concourse.bass as bass
    import concourse.mybir as mybir
    import concourse.tile as tile
    from concourse import bacc

    fp32 = mybir.dt.float32
    f32r = mybir.dt.float32r
    DT = mybir.dt.bfloat16      # dram/lhs/rhs matmul dtype
    SDT = mybir.dt.bfloat16     # on-chip storage for q/k/v/p/attn
    CH = 512                    # token chunk for phase 1

    def mm(ap):
        return ap.bitcast(DT) if DT is not ap.dtype else ap

    causal = variant == "causal"
    NCH = S // CH            # phase-1 token chunks
    NT = S // 128            # 16 token tiles
    NPR = 4                  # head pairs

    nc = bacc.Bacc("TRN2", target_bir_lowering=False, debug=False,
                   enable_asserts=False, num_devices=N_CORES)

    xT = nc.dram_tensor("xT", [HID, S], DT, kind="ExternalInput").ap()
    wqT = nc.dram_tensor("wqT", [HID, BLK], DT, kind="ExternalInput").ap()
    wkT = nc.dram_tensor("wkT", [HID, BLK], DT, kind="ExternalInput").ap()
    wvT = nc.dram_tensor("wvT", [HID, BLK], DT, kind="ExternalInput").ap()
    woT = nc.dram_tensor("woT", [BLK, HID], DT, kind="ExternalInput").ap()
    cos2 = nc.dram_tensor("cos2", [128, S], DT, kind="ExternalInput").ap()
    sin2s = nc.dram_tensor("sin2s", [128, S], DT, kind="ExternalInput").ap()
    if not causal:
        maskT = nc.dram_tensor("maskT", [S, S], mybir.dt.bfloat16,
                               kind="ExternalInput").ap()
    out = nc.dram_tensor("out", [S, HID], fp32, kind="ExternalOutput").ap()

    tc_ctx = tile.TileContext(nc)

    def phase1(tc, qT_sb, kT_sb, v_sb):
        with tc.tile_pool(name="consts", bufs=1) as cpool, \
             tc.tile_pool(name="wall", bufs=1) as wpool, \
             tc.tile_pool(name="xa", bufs=2) as xa, \
             tc.tile_pool(name="ropetmp", bufs=2) as rt, \
             tc.tile_pool(name="ps1", bufs=8, space="PSUM") as ps1:
            # DMA emission order matters for startup latency: the first
            # matmul group needs x chunk 0 + wq only; cos/sin only matter
            # ~a dozen matmuls later; wk/wv are emitted mid-chunk-0 so
            # they don't steal HBM bandwidth from the critical prefetch.
            wq_sb = wpool.tile([128, 16, BLK], DT)
            wk_sb = wpool.tile([128, 16, BLK], DT)
            wv_sb = wpool.tile([128, 16, BLK], DT)
            cos_sb = cpool.tile([128, S], DT)
            sin_sb = cpool.tile([128, S], DT)

            def dma_w(w_sb, w_d):
                wd = w_d.rearrange("(k p) m -> p k m", p=128)
                for kg in range(4):
                    nc.sync.dma_start(out=w_sb[:, 4 * kg:4 * kg + 4, :],
                                      in_=wd[:, 4 * kg:4 * kg + 4, :])

            x_tiles = {}

            def dma_x(n):
                sl = slice(n * CH, (n + 1) * CH)
                x_ch = xa.tile([128, 16, CH], DT, name=f"x_{n}", tag="x")
                xd = xT[:, sl].rearrange("(k p) t -> p k t", p=128)
                for kg in range(4):
                    nc.sync.dma_start(out=x_ch[:, 4 * kg:4 * kg + 4, :],
                                      in_=xd[:, 4 * kg:4 * kg + 4, :])
                x_tiles[n] = x_ch

            dma_x(0)
            dma_w(wq_sb, wqT)
            nc.sync.dma_start(out=cos_sb, in_=cos2)
            nc.sync.dma_start(out=sin_sb, in_=sin2s)

            for n in range(NCH):
                sl = slice(n * CH, (n + 1) * CH)
                if n not in x_tiles:
                    dma_x(n)
                x_ch = x_tiles.pop(n)
                # --- Q/K + RoPE ---
                for w_sb, dst, wn in ((wq_sb, qT_sb, "q"), (wk_sb, kT_sb, "k")):
                    for m in range(NPR):
                        ps = ps1.tile([128, CH], fp32,
                                      name=f"ps{wn}{m}_{n}", tag="ps1")
                        for k in range(16):
                            nc.tensor.matmul(
                                ps,
                                lhsT=mm(w_sb[:, k, m * 128:(m + 1) * 128]),
                                rhs=mm(x_ch[:, k, :]),
                                start=(k == 0), stop=(k == 15))
                        # rot = partition-shift of ps via 4 ACT copies
                        # (PSUM source, so the SBUF same-start rule only
                        # sees the output), then 2 full-width DVE muls +
                        # GPS add.  Spreads rope across ACT/DVE/GPS.
                        rot = rt.tile([128, CH], fp32, name=f"ro{wn}{m}_{n}", tag="ro")
                        # last chunk's shifts go on DVE so the phase-2 exps
                        # don't queue behind an ACT backlog at the boundary
                        shift_eng = nc.vector if n == NCH - 1 else nc.scalar
                        for (d, s_) in ((0, 32), (32, 0), (64, 96), (96, 64)):
                            if shift_eng is nc.scalar:
                                nc.scalar.copy(rot[d:d + 32, :], ps[s_:s_ + 32, :])
                            else:
                                nc.vector.tensor_copy(rot[d:d + 32, :],
                                                      ps[s_:s_ + 32, :])
                        t1 = rt.tile([128, CH], fp32, name=f"t1{wn}{m}_{n}", tag="t1")
                        t2 = rt.tile([128, CH], fp32, name=f"t2{wn}{m}_{n}", tag="t2")
                        nc.vector.tensor_mul(t1, ps, cos_sb[:, sl])
                        nc.vector.tensor_mul(t2, rot, sin_sb[:, sl])
                        nc.gpsimd.tensor_add(dst[:, m, sl], t1, t2)
                    if n == 0 and wn == "q":
                        dma_w(wk_sb, wkT)      # after Q's prefetch drained
                if n == 0:
                    dma_w(wv_sb, wvT)
                    dma_x(1)
                # --- V (same x chunk) ---
                for s_ in range(CH // 128):
                    t16 = (n * CH) // 128 + s_
                    ps = ps1.tile([128, BLK], fp32, name=f"psv{t16}", tag="ps1")
                    for k in range(16):
                        nc.tensor.matmul(
                            ps,
                            lhsT=mm(x_ch[:, k, s_ * 128:(s_ + 1) * 128]),
                            rhs=mm(wv_sb[:, k, :]),
                            start=(k == 0), stop=(k == 15))
                    dstv = v_sb[:, t16, :].rearrange("p (h c) -> p h c", c=65)[:, :, 0:64]
                    nc.vector.tensor_copy(dstv, ps.rearrange("p (h c) -> p h c", c=64))

    with tc_ctx as tc:
        with tc.tile_pool(name="qk_sb", bufs=1) as qk_pool, \
             tc.tile_pool(name="v_sb_pool", bufs=1) as v_pool:
            qT_sb = qk_pool.tile([128, NPR, S], SDT)
            kT_sb = qk_pool.tile([128, NPR, S], SDT)
            v_sb = v_pool.tile([128, NT, HEADS_PER_CORE * 65], SDT)
            ones_ap = v_sb.rearrange("p t (h c) -> p t h c", c=65)[:, :, :, 64:65]
            nc.vector.memset(ones_ap, 1.0)

            phase1(tc, qT_sb, kT_sb, v_sb)

            # -------- phase 2+3: attention + fused out-projection ----
            with tc.tile_pool(name="attn_pool", bufs=1) as apool, \
                 tc.tile_pool(name="wo", bufs=1) as wop:
                attn_sb = apool.tile([128, NPR, S], SDT)
                wo_sb = wop.tile([128, NPR, HID], DT)
                nc.sync.dma_start(out=wo_sb,
                                  in_=woT.rearrange("(r p) o -> p r o", p=128))
                with tc.tile_pool(name="ppool", bufs=3) as ppool, \
                     tc.tile_pool(name="npool", bufs=2) as npool, \
                     tc.tile_pool(name="mpool", bufs=2) as mpool, \
                     tc.tile_pool(name="ostage", bufs=2) as ostage, \
                     tc.tile_pool(name="pss", bufs=2, space="PSUM") as pss, \
                     tc.tile_pool(name="psav", bufs=1, space="PSUM") as psav, \
                     tc.tile_pool(name="opool", bufs=2, space="PSUM") as opool:

                    from collections import deque
                    pending_out = deque()

                    def emit_outproj_group(t16, no):
                        o_ps = opool.tile([128, 512], fp32,
                                          name=f"o{t16}_{no}", tag="o")
                        for pr_ in range(NPR):
                            nc.tensor.matmul(
                                o_ps,
                                lhsT=mm(attn_sb[:, pr_,
                                                t16 * 128:(t16 + 1) * 128]),
                                rhs=mm(wo_sb[:, pr_,
                                             no * 512:(no + 1) * 512]),
                                start=(pr_ == 0), stop=(pr_ == NPR - 1))
                        o_sb = ostage.tile([128, 512], fp32,
                                           name=f"os{t16}_{no}", tag="os")
                        nc.vector.tensor_copy(o_sb, o_ps)
                        nc.sync.dma_start(
                            out=out[t16 * 128:(t16 + 1) * 128,
                                    no * 512:(no + 1) * 512],
                            in_=o_sb)

                    def drain_one():
                        if pending_out:
                            t16, no = pending_out.popleft()
                            emit_outproj_group(t16, no)

                    col_iter = [0]

                    def attend(pr, j, mask_col, pace=1):
                        n_i = 4 * j + 4 if causal else NT
                        # full-bank AV tiles: rows 0-64 accumulate
                        # (out.T | denom); rows 64-127 later hold the
                        # PE-broadcast reciprocal
                        av = [psav.tile([128, 512], fp32,
                                        name=f"av{half}_{pr}_{j}", tag=f"av{half}")
                              for half in range(2)]
                        s_t, p_t, nw_t = {}, {}, {}

                        def emit_S(i):
                            # causal boundary tiles only need the tq
                            # suffix [512j+off, 512(j+1))
                            off = max(0, 128 * (i - 4 * j)) if causal else 0
                            nw = 512 - off
                            nw_t[i] = (off, nw)
                            s_ps = pss.tile([128, 1024], fp32,
                                            name=f"s_{pr}_{j}_{i}", tag="s")
                            for half in range(2):
                                r0 = 64 * half
                                nc.tensor.matmul(
                                    s_ps[:, half * 512:half * 512 + nw],
                                    lhsT=mm(kT_sb[r0:r0 + 64, pr,
                                                  i * 128:(i + 1) * 128]),
                                    rhs=mm(qT_sb[r0:r0 + 64, pr,
                                                 j * 512 + off:(j + 1) * 512]),
                                    start=True, stop=True,
                                    tile_position=(r0, 0))
                            s_t[i] = s_ps

                        def emit_exp(i):
                            off, nw = nw_t[i]
                            s_ps = s_t[i]
                            s_v = s_ps.rearrange("q (h t) -> q h t", h=2)[:, :, 0:nw]
                            p = ppool.tile([128, 1024], SDT,
                                           name=f"p_{pr}_{j}_{i}", tag="p")
                            p_v = p.rearrange("q (h t) -> q h t", h=2)[:, :, 0:nw]
                            if causal:
                                nc.scalar.activation(p_v, s_v,
                                                     mybir.ActivationFunctionType.Exp,
                                                     scale=0.125)
                                if i >= 4 * j:
                                    # keep iff tq - tk >= 0 (base is 0 on
                                    # boundary tiles thanks to the suffix)
                                    nc.gpsimd.affine_select(
                                        out=p_v, in_=p_v,
                                        compare_op=mybir.AluOpType.is_ge,
                                        fill=0.0,
                                        base=512 * j + off - 128 * i,
                                        pattern=[[0, 2], [1, nw]],
                                        channel_multiplier=-1)
                            else:
                                tmp = ppool.tile([128, 1024], fp32,
                                                 name=f"pt_{pr}_{j}_{i}", tag="pt")
                                for half in range(2):
                                    nc.vector.scalar_tensor_tensor(
                                        out=tmp[:, half * 512:(half + 1) * 512],
                                        in0=s_ps[:, half * 512:(half + 1) * 512],
                                        scalar=0.125,
                                        in1=mask_col[:, i, :],
                                        op0=mybir.AluOpType.mult,
                                        op1=mybir.AluOpType.add)
                                nc.scalar.activation(p, tmp,
                                                     mybir.ActivationFunctionType.Exp)
                            p_t[i] = p

                        def emit_AV(i, first, last):
                            off, nw = nw_t[i]
                            p = p_t[i]
                            for half in range(2):
                                h = 2 * pr + half
                                nc.tensor.matmul(
                                    av[half][0:65, off:512],
                                    lhsT=mm(v_sb[:, i, 65 * h:65 * h + 65]),
                                    rhs=mm(p[:, half * 512:half * 512 + nw]),
                                    start=first, stop=last)

                        # software pipeline: S runs 2 tiles ahead of AV,
                        # with outproj matmul groups drained between
                        # iterations as PE filler while ACT runs exp.
                        # (ascending i is required: AV i=0 writes the full
                        # 512 with start=True, clearing has_written before
                        # the suffix-trimmed boundary tiles accumulate)
                        order = list(range(n_i))
                        first_i, last_i = order[0], order[-1]
                        emit_S(order[0])
                        if n_i > 1:
                            emit_S(order[1])
                        emit_exp(order[0])
                        for ii, i in enumerate(order):
                            if ii + 2 < n_i:
                                emit_S(order[ii + 2])
                            if ii + 1 < n_i:
                                emit_exp(order[ii + 1])
                            # drain BEFORE AV: the outproj group gives PE
                            # ready work to cover the exp->AV semaphore
                            # latency (PE is FIFO; AV would head-of-line
                            # block an already-ready outproj group).
                            # Paced so the queue lasts the whole column.
                            col_iter[0] += 1
                            if col_iter[0] % pace == 0:
                                drain_one()
                            emit_AV(i, i == first_i, i == last_i)

                        # normalize rows 0..63 by row 64: fast approximate
                        # reciprocal on DVE (~51 ULP, plenty for a softmax
                        # denominator), partition-broadcast on GPSIMD, then
                        # one DVE multiply into attn_sb.  Keeps the PE
                        # stream free of ops that wait on DVE round-trips.
                        # emit both halves' DVE prep first, then the GPS
                        # broadcasts, then the muls — otherwise mul(half0)
                        # head-of-line blocks DVE while GPS broadcasts.
                        recs, bcrs = [], []
                        for half in range(2):
                            den = npool.tile([1, 512], fp32,
                                             name=f"den{half}_{pr}_{j}",
                                             tag=f"den{half}")
                            # approx recip is a bitwise custom-DVE op and
                            # must read SBUF, not PSUM — bounce the row.
                            nc.vector.tensor_copy(den, av[half][64:65, :])
                            rec = npool.tile([1, 512], fp32,
                                             name=f"rec{half}_{pr}_{j}",
                                             tag=f"rec{half}")
                            nc.vector.reciprocal_approx_fast(rec, den)
                            recs.append(rec)
                        for half in range(2):
                            bcr = npool.tile([64, 512], fp32,
                                             name=f"bc{half}_{pr}_{j}",
                                             tag=f"bc{half}")
                            nc.gpsimd.partition_broadcast(bcr, recs[half])
                            bcrs.append(bcr)
                        for half in range(2):
                            r0 = 64 * half
                            nc.vector.tensor_mul(
                                attn_sb[r0:r0 + 64, pr, j * 512:(j + 1) * 512],
                                av[half][0:64, :], bcrs[half])

                    for j in range(4):
                        if causal:
                            mask_col = None
                        else:
                            mask_col = mpool.tile([128, NT, 512],
                                                  mybir.dt.bfloat16,
                                                  name=f"mc{j}", tag="mc")
                            nc.sync.dma_start(
                                out=mask_col,
                                in_=maskT[:, j * 512:(j + 1) * 512]
                                .rearrange("(i p) t -> p i t", p=128))
                        col_iter[0] = 0
                        for pr in range(NPR):
                            attend(pr, j, mask_col, pace=(j + 1) if causal else 4)
                        # column j's attention rows are complete; queue
                        # its out-projection as PE filler for column j+1
                        for t16 in range(4 * j, 4 * j + 4):
                            for no in range(4):
                                pending_out.append((t16, no))
                    while pending_out:
                        drain_one()
    nc.compile()
    return nc


def _get_program(variant):
    if variant not in _PROGRAMS:
        _PROGRAMS[variant] = build_program(variant)
    return _PROGRAMS[variant]


def make_in_maps(hidden_states, attention_mask, position_ids, Wq, Wk, Wv, Wo,
                 variant):
    npdt = ml_dtypes.bfloat16
    cos, sin = _rope_cache()
    in_maps = []
    for c in range(N_CORES):
        b = c // (N_CORES // 2)
        hb = c % (N_CORES // 2)
        rs = slice(hb * BLK, (hb + 1) * BLK)
        pos = np.asarray(position_ids[b]).astype(np.int64)
        cos_b = cos[pos].T.astype(np.float32)     # [64, S]
        sin_b = sin[pos].T.astype(np.float32)
        sin_s = np.concatenate([-sin_b[:32], sin_b[32:]], axis=0)
        m = {
            "xT": np.ascontiguousarray(np.asarray(hidden_states)[b].T).astype(npdt),
            "wqT": np.ascontiguousarray(np.asarray(Wq)[rs].T).astype(npdt),
            "wkT": np.ascontiguousarray(np.asarray(Wk)[rs].T).astype(npdt),
            "wvT": np.ascontiguousarray(np.asarray(Wv)[rs].T).astype(npdt),
            "woT": np.ascontiguousarray(np.asarray(Wo)[:, rs].T).astype(npdt),
            "cos2": np.ascontiguousarray(
                np.concatenate([cos_b, cos_b], axis=0)).astype(npdt),
            "sin2s": np.ascontiguousarray(
                np.concatenate([sin_s, sin_s], axis=0)).astype(npdt),
        }
        if variant == "general":
            m["maskT"] = np.ascontiguousarray(
                np.asarray(attention_mask)[b, 0].T).astype(ml_dtypes.bfloat16)
        in_maps.append(m)
    return in_maps


def detect_causal(attention_mask):
    am = np.asarray(attention_mask)
    if am.shape != (2, 1, S, S):
        return False
    neg = np.float32(np.finfo(np.float32).min)
    canonical = np.where(np.tril(np.ones((S, S), dtype=bool)), np.float32(0.0), neg)
    return bool(np.array_equal(am[0, 0], canonical) and
                np.array_equal(am[1, 0], canonical))


def kernel(hidden_states, attention_mask, position_ids, Wq, Wk, Wv, Wo):
    hidden_states = np.asarray(hidden_states, dtype=np.float32)
    attention_mask = np.asarray(attention_mask, dtype=np.float32)
    Wq, Wk, Wv, Wo = (np.asarray(w, dtype=np.float32) for w in (Wq, Wk, Wv, Wo))

    variant = "causal" if detect_causal(attention_mask) else "general"
    nc = _get_program(variant)
    in_maps = make_in_maps(hidden_states, attention_mask, position_ids,
                           Wq, Wk, Wv, Wo, variant)

    from concourse import bass2jax
    results = bass2jax.run_bass_via_pjrt(nc, in_maps, n_cores=N_CORES)

    out = np.zeros((2, S, HID), dtype=np.float64)
    for c in range(N_CORES):
        b = c // (N_CORES // 2)
        out[b] += results[c]["out"].astype(np.float64)
    return out.astype(np.float32)


# revision 25
# speedup vs baseline: 1.3044x; 1.0267x over previous
"""MiniCPM attention block on 8 Trainium2 NeuronCores.

Sharding: core c handles batch b = c // 4 and the 8 heads
[ (c%4)*8, (c%4)*8 + 8 ) of that batch (tensor-parallel over heads +
data-parallel over batch).  Each core computes a partial output
x @ block-of-Wo.T of shape [S, HID]; the host sums the 4 partials per
batch.  No collectives.

Device pipeline per core (S=2048 tokens, 8 heads of d=64):
  1. per 512-token chunk: qT/kT = (x @ Wq_s.T).T with RoPE fused into
     the PSUM->SBUF evict, and v = x @ Wv_s.T (stored [tk, 8*65] with a
     ones column per head) — one x load per chunk shared by Q/K/V, all
     12 PSUM groups rotating through one 8-slot pool.
  2. per head-pair, per tq-chunk j (512): S.T tiles [tk 128, tq 512]
     on PE (2 heads packed via row tiling -> concurrent), exp on ACT
     (pair-fused into one [128,1024] op), causal zeroing via gpsimd
     affine_select, AV accumulation on PE with the ones column giving
     the softmax denominator for free.  Normalize via DVE reciprocal +
     a K=1 PE broadcast matmul (ones^T @ rec -> psum rows 64:128) +
     DVE copy/mul — no DRAM bounce.  The i-loop is software-pipelined
     (S two tiles ahead of AV) so PE never waits on the ACT exp
     round-trip.  Causal boundary tiles compute only the valid tq
     suffix.
  3. out_partial = attn_outT.T @ Wo_s.T in [128,512] PSUM groups with
     DVE evicts, interleaved per tq-column so PE has dense work while
     ACT runs the next column's exps.

Self-contained: shapes hardcoded from the problem spec.
"""
import numpy as np
import ml_dtypes

S = 2048
HID = 2048
NH = 32
DH = 64
N_CORES = 8
HEADS_PER_CORE = NH // (N_CORES // 2)   # 8
BLK = HEADS_PER_CORE * DH               # 512
ROPE_BASE = 10000.0

_PROGRAMS = {}


def _rope_cache():
    inv_freq = 1.0 / (ROPE_BASE ** (np.arange(0, DH, 2, dtype=np.float32) / DH))
    t = np.arange(S, dtype=np.float32)
    freqs = np.outer(t, inv_freq)                     # [S, 32]
    emb = np.concatenate([freqs, freqs], axis=-1)     # [S, 64]
    return np.cos(emb), np.sin(emb)


def build_program(variant="causal"):
    """Build the Bacc program (one NEFF, run SPMD on 8 cores)."""
    import concourse.bass as bass
    import concourse.mybir as mybir
    import concourse.tile as tile
    from concourse import bacc

    fp32 = mybir.dt.float32
    f32r = mybir.dt.float32r
    DT = mybir.dt.bfloat16      # dram/lhs/rhs matmul dtype
    SDT = mybir.dt.bfloat16     # on-chip storage for q/k/v/p/attn
    CH = 512                    # token chunk for phase 1

    def mm(ap):
        return ap.bitcast(DT) if DT is not ap.dtype else ap

    causal = variant == "causal"
    NCH = S // CH            # phase-1 token chunks
    NT = S // 128            # 16 token tiles
    NPR = 4                  # head pairs

    nc = bacc.Bacc("TRN2", target_bir_lowering=False, debug=False,
                   enable_asserts=False, num_devices=N_CORES)

    xT = nc.dram_tensor("xT", [HID, S], DT, kind="ExternalInput").ap()
    wqT = nc.dram_tensor("wqT", [HID, BLK], DT, kind="ExternalInput").ap()
    wkT = nc.dram_tensor("wkT", [HID, BLK], DT, kind="ExternalInput").ap()
    wvT = nc.dram_tensor("wvT", [HID, BLK], DT, kind="ExternalInput").ap()
    woT = nc.dram_tensor("woT", [BLK, HID], DT, kind="ExternalInput").ap()
    cos2 = nc.dram_tensor("cos2", [128, S], DT, kind="ExternalInput").ap()
    sin2s = nc.dram_tensor("sin2s", [128, S], DT, kind="ExternalInput").ap()
    if not causal:
        maskT = nc.dram_tensor("maskT", [S, S], mybir.dt.bfloat16,
                               kind="ExternalInput").ap()
    out = nc.dram_tensor("out", [S, HID], fp32, kind="ExternalOutput").ap()

    tc_ctx = tile.TileContext(nc)

    def phase1(tc, qT_sb, kT_sb, v_sb):
        with tc.tile_pool(name="consts", bufs=1) as cpool, \
             tc.tile_pool(name="wall", bufs=1) as wpool, \
             tc.tile_pool(name="xa", bufs=2) as xa, \
             tc.tile_pool(name="ropetmp", bufs=2) as rt, \
             tc.tile_pool(name="ps1", bufs=8, space="PSUM") as ps1:
            # DMA emission order matters for startup latency: the first
            # matmul group needs x chunk 0 + wq only; cos/sin only matter
            # ~a dozen matmuls later; wk/wv are emitted mid-chunk-0 so
            # they don't steal HBM bandwidth from the critical prefetch.
            wq_sb = wpool.tile([128, 16, BLK], DT)
            wk_sb = wpool.tile([128, 16, BLK], DT)
            wv_sb = wpool.tile([128, 16, BLK], DT)
            cos_sb = cpool.tile([128, S], DT)
            sin_sb = cpool.tile([128, S], DT)

            def dma_w(w_sb, w_d):
                wd = w_d.rearrange("(k p) m -> p k m", p=128)
                for kg in range(4):
                    nc.sync.dma_start(out=w_sb[:, 4 * kg:4 * kg + 4, :],
                                      in_=wd[:, 4 * kg:4 * kg + 4, :])

            x_tiles = {}

            def dma_x(n):
                sl = slice(n * CH, (n + 1) * CH)
                x_ch = xa.tile([128, 16, CH], DT, name=f"x_{n}", tag="x")
                xd = xT[:, sl].rearrange("(k p) t -> p k t", p=128)
                for kg in range(4):
                    nc.sync.dma_start(out=x_ch[:, 4 * kg:4 * kg + 4, :],
                                      in_=xd[:, 4 * kg:4 * kg + 4, :])
                x_tiles[n] = x_ch

            dma_x(0)
            dma_w(wq_sb, wqT)
            nc.sync.dma_start(out=cos_sb, in_=cos2)
            nc.sync.dma_start(out=sin_sb, in_=sin2s)

            for n in range(NCH):
                sl = slice(n * CH, (n + 1) * CH)
                if n not in x_tiles:
                    dma_x(n)
                x_ch = x_tiles.pop(n)
                # --- Q/K + RoPE ---
                for w_sb, dst, wn in ((wq_sb, qT_sb, "q"), (wk_sb, kT_sb, "k")):
                    for m in range(NPR):
                        ps = ps1.tile([128, CH], fp32,
                                      name=f"ps{wn}{m}_{n}", tag="ps1")
                        for k in range(16):
                            nc.tensor.matmul(
                                ps,
                                lhsT=mm(w_sb[:, k, m * 128:(m + 1) * 128]),
                                rhs=mm(x_ch[:, k, :]),
                                start=(k == 0), stop=(k == 15))
                        # rot = partition-shift of ps via 4 ACT copies
                        # (PSUM source, so the SBUF same-start rule only
                        # sees the output), then 2 full-width DVE muls +
                        # GPS add.  Spreads rope across ACT/DVE/GPS.
                        rot = rt.tile([128, CH], fp32, name=f"ro{wn}{m}_{n}", tag="ro")
                        # last chunk's shifts go on DVE so the phase-2 exps
                        # don't queue behind an ACT backlog at the boundary
                        shift_eng = nc.vector if n == NCH - 1 else nc.scalar
                        for (d, s_) in ((0, 32), (32, 0), (64, 96), (96, 64)):
                            if shift_eng is nc.scalar:
                                nc.scalar.copy(rot[d:d + 32, :], ps[s_:s_ + 32, :])
                            else:
                                nc.vector.tensor_copy(rot[d:d + 32, :],
                                                      ps[s_:s_ + 32, :])
                        t1 = rt.tile([128, CH], fp32, name=f"t1{wn}{m}_{n}", tag="t1")
                        t2 = rt.tile([128, CH], fp32, name=f"t2{wn}{m}_{n}", tag="t2")
                        nc.vector.tensor_mul(t1, ps, cos_sb[:, sl])
                        nc.vector.tensor_mul(t2, rot, sin_sb[:, sl])
                        nc.gpsimd.tensor_add(dst[:, m, sl], t1, t2)
                    if n == 0 and wn == "q":
                        dma_w(wk_sb, wkT)      # after Q's prefetch drained
                if n == 0:
                    dma_w(wv_sb, wvT)
                    dma_x(1)
                # --- V (same x chunk) ---
                for s_ in range(CH // 128):
                    t16 = (n * CH) // 128 + s_
                    ps = ps1.tile([128, BLK], fp32, name=f"psv{t16}", tag="ps1")
                    for k in range(16):
                        nc.tensor.matmul(
                            ps,
                            lhsT=mm(x_ch[:, k, s_ * 128:(s_ + 1) * 128]),
                            rhs=mm(wv_sb[:, k, :]),
                            start=(k == 0), stop=(k == 15))
                    dstv = v_sb[:, t16, :].rearrange("p (h c) -> p h c", c=65)[:, :, 0:64]
                    nc.vector.tensor_copy(dstv, ps.rearrange("p (h c) -> p h c", c=64))

    with tc_ctx as tc:
        with tc.tile_pool(name="qk_sb", bufs=1) as qk_pool, \
             tc.tile_pool(name="v_sb_pool", bufs=1) as v_pool:
            qT_sb = qk_pool.tile([128, NPR, S], SDT)
            kT_sb = qk_pool.tile([128, NPR, S], SDT)
            v_sb = v_pool.tile([128, NT, HEADS_PER_CORE * 65], SDT)
            ones_ap = v_sb.rearrange("p t (h c) -> p t h c", c=65)[:, :, :, 64:65]
            nc.vector.memset(ones_ap, 1.0)
            # warm the GPSIMD ucode library for partition_broadcast during
            # the initial DMA wait — the first use otherwise pays a ~7us
            # library fetch right on the phase-2 critical path
            with tc.tile_pool(name="warm", bufs=1) as warmp:
                wsrc = warmp.tile([1, 64], fp32)
                wdst = warmp.tile([2, 64], fp32)
                nc.vector.memset(wsrc, 1.0)
                nc.gpsimd.partition_broadcast(wdst, wsrc)

            phase1(tc, qT_sb, kT_sb, v_sb)

            # -------- phase 2+3: attention + fused out-projection ----
            with tc.tile_pool(name="attn_pool", bufs=1) as apool, \
                 tc.tile_pool(name="wo", bufs=1) as wop:
                attn_sb = apool.tile([128, NPR, S], SDT)
                wo_sb = wop.tile([128, NPR, HID], DT)
                nc.sync.dma_start(out=wo_sb,
                                  in_=woT.rearrange("(r p) o -> p r o", p=128))
                with tc.tile_pool(name="ppool", bufs=4) as ppool, \
                     tc.tile_pool(name="npool", bufs=2) as npool, \
                     tc.tile_pool(name="mpool", bufs=2) as mpool, \
                     tc.tile_pool(name="ostage", bufs=2) as ostage, \
                     tc.tile_pool(name="pss", bufs=2, space="PSUM") as pss, \
                     tc.tile_pool(name="psav", bufs=1, space="PSUM") as psav, \
                     tc.tile_pool(name="opool", bufs=2, space="PSUM") as opool:

                    from collections import deque
                    pending_out = deque()

                    def emit_outproj_group(t16, no):
                        o_ps = opool.tile([128, 512], fp32,
                                          name=f"o{t16}_{no}", tag="o")
                        for pr_ in range(NPR):
                            nc.tensor.matmul(
                                o_ps,
                                lhsT=mm(attn_sb[:, pr_,
                                                t16 * 128:(t16 + 1) * 128]),
                                rhs=mm(wo_sb[:, pr_,
                                             no * 512:(no + 1) * 512]),
                                start=(pr_ == 0), stop=(pr_ == NPR - 1))
                        o_sb = ostage.tile([128, 512], fp32,
                                           name=f"os{t16}_{no}", tag="os")
                        nc.vector.tensor_copy(o_sb, o_ps)
                        nc.sync.dma_start(
                            out=out[t16 * 128:(t16 + 1) * 128,
                                    no * 512:(no + 1) * 512],
                            in_=o_sb)

                    def drain_one():
                        if pending_out:
                            t16, no = pending_out.popleft()
                            emit_outproj_group(t16, no)

                    col_iter = [0]

                    def attend(pr, j, mask_col, pace=1):
                        n_i = 4 * j + 4 if causal else NT
                        # full-bank AV tiles: rows 0-64 accumulate
                        # (out.T | denom); rows 64-127 later hold the
                        # PE-broadcast reciprocal
                        av = [psav.tile([128, 512], fp32,
                                        name=f"av{half}_{pr}_{j}", tag=f"av{half}")
                              for half in range(2)]
                        s_t, p_t, nw_t = {}, {}, {}

                        def emit_S(i):
                            # causal boundary tiles only need the tq
                            # suffix [512j+off, 512(j+1))
                            off = max(0, 128 * (i - 4 * j)) if causal else 0
                            nw = 512 - off
                            nw_t[i] = (off, nw)
                            s_ps = pss.tile([128, 1024], fp32,
                                            name=f"s_{pr}_{j}_{i}", tag="s")
                            for half in range(2):
                                r0 = 64 * half
                                nc.tensor.matmul(
                                    s_ps[:, half * 512:half * 512 + nw],
                                    lhsT=mm(kT_sb[r0:r0 + 64, pr,
                                                  i * 128:(i + 1) * 128]),
                                    rhs=mm(qT_sb[r0:r0 + 64, pr,
                                                 j * 512 + off:(j + 1) * 512]),
                                    start=True, stop=True,
                                    tile_position=(r0, 0))
                            s_t[i] = s_ps

                        def emit_exp(i):
                            off, nw = nw_t[i]
                            s_ps = s_t[i]
                            s_v = s_ps.rearrange("q (h t) -> q h t", h=2)[:, :, 0:nw]
                            p = ppool.tile([128, 1024], SDT,
                                           name=f"p_{pr}_{j}_{i}", tag="p")
                            p_v = p.rearrange("q (h t) -> q h t", h=2)[:, :, 0:nw]
                            if causal:
                                nc.scalar.activation(p_v, s_v,
                                                     mybir.ActivationFunctionType.Exp,
                                                     scale=0.125)
                                if i >= 4 * j:
                                    # keep iff tq - tk >= 0 (base is 0 on
                                    # boundary tiles thanks to the suffix)
                                    nc.gpsimd.affine_select(
                                        out=p_v, in_=p_v,
                                        compare_op=mybir.AluOpType.is_ge,
                                        fill=0.0,
                                        base=512 * j + off - 128 * i,
                                        pattern=[[0, 2], [1, nw]],
                                        channel_multiplier=-1)
                            else:
                                tmp = ppool.tile([128, 1024], fp32,
                                                 name=f"pt_{pr}_{j}_{i}", tag="pt")
                                for half in range(2):
                                    nc.vector.scalar_tensor_tensor(
                                        out=tmp[:, half * 512:(half + 1) * 512],
                                        in0=s_ps[:, half * 512:(half + 1) * 512],
                                        scalar=0.125,
                                        in1=mask_col[:, i, :],
                                        op0=mybir.AluOpType.mult,
                                        op1=mybir.AluOpType.add)
                                nc.scalar.activation(p, tmp,
                                                     mybir.ActivationFunctionType.Exp)
                            p_t[i] = p

                        def emit_AV(i, first, last):
                            off, nw = nw_t[i]
                            p = p_t[i]
                            for half in range(2):
                                h = 2 * pr + half
                                nc.tensor.matmul(
                                    av[half][0:65, off:512],
                                    lhsT=mm(v_sb[:, i, 65 * h:65 * h + 65]),
                                    rhs=mm(p[:, half * 512:half * 512 + nw]),
                                    start=first, stop=last)

                        # software pipeline: S runs 2 tiles ahead of AV,
                        # with outproj matmul groups drained between
                        # iterations as PE filler while ACT runs exp.
                        # (ascending i is required: AV i=0 writes the full
                        # 512 with start=True, clearing has_written before
                        # the suffix-trimmed boundary tiles accumulate)
                        order = list(range(n_i))
                        first_i, last_i = order[0], order[-1]
                        emit_S(order[0])
                        emit_exp(order[0])
                        if n_i > 1:
                            emit_S(order[1])
                            emit_exp(order[1])
                        for ii, i in enumerate(order):
                            # drain FIRST: the outproj group is the only PE
                            # work with no dependence on exp(ii) — S needs
                            # exp(ii)'s psum slot and AV needs p(ii) — so it
                            # must go ahead of them in PE's FIFO to cover
                            # the exp tail.  Paced so the queue lasts the
                            # whole column.
                            col_iter[0] += 1
                            if col_iter[0] % pace == 0:
                                drain_one()
                            if ii + 2 < n_i:
                                emit_S(order[ii + 2])
                                emit_exp(order[ii + 2])
                            emit_AV(i, i == first_i, i == last_i)

                        # normalize rows 0..63 by row 64: fast approximate
                        # reciprocal on DVE (~51 ULP, plenty for a softmax
                        # denominator), partition-broadcast on GPSIMD, then
                        # one DVE multiply into attn_sb.  Keeps the PE
                        # stream free of ops that wait on DVE round-trips.
                        # Evict av (rows 0-64) to SBUF first — one copy per
                        # half frees the PSUM slot so the next pair's AV
                        # never waits on the rest of the normalize chain.
                        # Then approx-reciprocal (bitwise custom-DVE op,
                        # needs an SBUF source), GPS partition-broadcast,
                        # and one mul.  u/bcr are full-height tiles so the
                        # two SBUF inputs of the mul share base partition 0.
                        us, recs, bcrs = [], [], []
                        for half in range(2):
                            u = npool.tile([128, 512], fp32,
                                           name=f"u{half}_{pr}_{j}",
                                           tag=f"u{half}")
                            nc.vector.tensor_copy(u[0:65, :], av[half][0:65, :])
                            us.append(u)
                        for half in range(2):
                            # the approx-recip custom-DVE op NaNs when its
                            # input AP starts at a nonzero partition —
                            # bounce the denominator row to partition 0
                            den = npool.tile([1, 512], fp32,
                                             name=f"den{half}_{pr}_{j}",
                                             tag=f"den{half}")
                            nc.vector.tensor_copy(den, us[half][64:65, :])
                            rec = npool.tile([1, 512], fp32,
                                             name=f"rec{half}_{pr}_{j}",
                                             tag=f"rec{half}")
                            nc.vector.reciprocal_approx_fast(rec, den)
                            recs.append(rec)
                        for half in range(2):
                            bcr = npool.tile([128, 512], fp32,
                                             name=f"bc{half}_{pr}_{j}",
                                             tag=f"bc{half}")
                            nc.gpsimd.partition_broadcast(bcr[0:64, :], recs[half])
                            bcrs.append(bcr)
                        for half in range(2):
                            r0 = 64 * half
                            nc.vector.tensor_mul(
                                attn_sb[r0:r0 + 64, pr, j * 512:(j + 1) * 512],
                                us[half][0:64, :], bcrs[half][0:64, :])

                    for j in range(4):
                        if causal:
                            mask_col = None
                        else:
                            mask_col = mpool.tile([128, NT, 512],
                                                  mybir.dt.bfloat16,
                                                  name=f"mc{j}", tag="mc")
                            nc.sync.dma_start(
                                out=mask_col,
                                in_=maskT[:, j * 512:(j + 1) * 512]
                                .rearrange("(i p) t -> p i t", p=128))
                        col_iter[0] = 0
                        for pr in range(NPR):
                            attend(pr, j, mask_col, pace=(j + 1) if causal else 4)
                        # column j's attention rows are complete; queue
                        # its out-projection as PE filler for column j+1
                        for t16 in range(4 * j, 4 * j + 4):
                            for no in range(4):
                                pending_out.append((t16, no))
                    while pending_out:
                        drain_one()
    nc.compile()
    return nc


def _get_program(variant):
    if variant not in _PROGRAMS:
        _PROGRAMS[variant] = build_program(variant)
    return _PROGRAMS[variant]


def make_in_maps(hidden_states, attention_mask, position_ids, Wq, Wk, Wv, Wo,
                 variant):
    npdt = ml_dtypes.bfloat16
    cos, sin = _rope_cache()
    in_maps = []
    for c in range(N_CORES):
        b = c // (N_CORES // 2)
        hb = c % (N_CORES // 2)
        rs = slice(hb * BLK, (hb + 1) * BLK)
        pos = np.asarray(position_ids[b]).astype(np.int64)
        cos_b = cos[pos].T.astype(np.float32)     # [64, S]
        sin_b = sin[pos].T.astype(np.float32)
        sin_s = np.concatenate([-sin_b[:32], sin_b[32:]], axis=0)
        m = {
            "xT": np.ascontiguousarray(np.asarray(hidden_states)[b].T).astype(npdt),
            "wqT": np.ascontiguousarray(np.asarray(Wq)[rs].T).astype(npdt),
            "wkT": np.ascontiguousarray(np.asarray(Wk)[rs].T).astype(npdt),
            "wvT": np.ascontiguousarray(np.asarray(Wv)[rs].T).astype(npdt),
            "woT": np.ascontiguousarray(np.asarray(Wo)[:, rs].T).astype(npdt),
            "cos2": np.ascontiguousarray(
                np.concatenate([cos_b, cos_b], axis=0)).astype(npdt),
            "sin2s": np.ascontiguousarray(
                np.concatenate([sin_s, sin_s], axis=0)).astype(npdt),
        }
        if variant == "general":
            m["maskT"] = np.ascontiguousarray(
                np.asarray(attention_mask)[b, 0].T).astype(ml_dtypes.bfloat16)
        in_maps.append(m)
    return in_maps


def detect_causal(attention_mask):
    am = np.asarray(attention_mask)
    if am.shape != (2, 1, S, S):
        return False
    neg = np.float32(np.finfo(np.float32).min)
    canonical = np.where(np.tril(np.ones((S, S), dtype=bool)), np.float32(0.0), neg)
    return bool(np.array_equal(am[0, 0], canonical) and
                np.array_equal(am[1, 0], canonical))


def kernel(hidden_states, attention_mask, position_ids, Wq, Wk, Wv, Wo):
    hidden_states = np.asarray(hidden_states, dtype=np.float32)
    attention_mask = np.asarray(attention_mask, dtype=np.float32)
    Wq, Wk, Wv, Wo = (np.asarray(w, dtype=np.float32) for w in (Wq, Wk, Wv, Wo))

    variant = "causal" if detect_causal(attention_mask) else "general"
    nc = _get_program(variant)
    in_maps = make_in_maps(hidden_states, attention_mask, position_ids,
                           Wq, Wk, Wv, Wo, variant)

    from concourse import bass2jax
    results = bass2jax.run_bass_via_pjrt(nc, in_maps, n_cores=N_CORES)

    out = np.zeros((2, S, HID), dtype=np.float64)
    for c in range(N_CORES):
        b = c // (N_CORES // 2)
        out[b] += results[c]["out"].astype(np.float64)
    return out.astype(np.float32)


# revision 28
# speedup vs baseline: 1.3096x; 1.0040x over previous
"""MiniCPM attention block on 8 Trainium2 NeuronCores.

Sharding: core c handles batch b = c // 4 and the 8 heads
[ (c%4)*8, (c%4)*8 + 8 ) of that batch (tensor-parallel over heads +
data-parallel over batch).  Each core computes a partial output
x @ block-of-Wo.T of shape [S, HID]; the host sums the 4 partials per
batch.  No collectives.

Device pipeline per core (S=2048 tokens, 8 heads of d=64):
  1. per 512-token chunk: qT/kT = (x @ Wq_s.T).T with RoPE fused into
     the PSUM->SBUF evict, and v = x @ Wv_s.T (stored [tk, 8*65] with a
     ones column per head) — one x load per chunk shared by Q/K/V, all
     12 PSUM groups rotating through one 8-slot pool.
  2. per head-pair, per tq-chunk j (512): S.T tiles [tk 128, tq 512]
     on PE (2 heads packed via row tiling -> concurrent), exp on ACT
     (pair-fused into one [128,1024] op), causal zeroing via gpsimd
     affine_select, AV accumulation on PE with the ones column giving
     the softmax denominator for free.  Normalize via DVE reciprocal +
     a K=1 PE broadcast matmul (ones^T @ rec -> psum rows 64:128) +
     DVE copy/mul — no DRAM bounce.  The i-loop is software-pipelined
     (S two tiles ahead of AV) so PE never waits on the ACT exp
     round-trip.  Causal boundary tiles compute only the valid tq
     suffix.
  3. out_partial = attn_outT.T @ Wo_s.T in [128,512] PSUM groups with
     DVE evicts, interleaved per tq-column so PE has dense work while
     ACT runs the next column's exps.

Self-contained: shapes hardcoded from the problem spec.
"""
import numpy as np
import ml_dtypes

S = 2048
HID = 2048
NH = 32
DH = 64
N_CORES = 8
HEADS_PER_CORE = NH // (N_CORES // 2)   # 8
BLK = HEADS_PER_CORE * DH               # 512
ROPE_BASE = 10000.0

_PROGRAMS = {}


def _rope_cache():
    inv_freq = 1.0 / (ROPE_BASE ** (np.arange(0, DH, 2, dtype=np.float32) / DH))
    t = np.arange(S, dtype=np.float32)
    freqs = np.outer(t, inv_freq)                     # [S, 32]
    emb = np.concatenate([freqs, freqs], axis=-1)     # [S, 64]
    return np.cos(emb), np.sin(emb)


def build_program(variant="causal"):
    """Build the Bacc program (one NEFF, run SPMD on 8 cores)."""
    import concourse.bass as bass
    import concourse.mybir as mybir
    import concourse.tile as tile
    from concourse import bacc

    fp32 = mybir.dt.float32
    f32r = mybir.dt.float32r
    DT = mybir.dt.bfloat16      # dram/lhs/rhs matmul dtype
    SDT = mybir.dt.bfloat16     # on-chip storage for q/k/v/p/attn
    CH = 512                    # token chunk for phase 1

    def mm(ap):
        return ap.bitcast(DT) if DT is not ap.dtype else ap

    causal = variant == "causal"
    NCH = S // CH            # phase-1 token chunks
    NT = S // 128            # 16 token tiles
    NPR = 4                  # head pairs

    nc = bacc.Bacc("TRN2", target_bir_lowering=False, debug=False,
                   enable_asserts=False, num_devices=N_CORES)

    xT = nc.dram_tensor("xT", [HID, S], DT, kind="ExternalInput").ap()
    wqT = nc.dram_tensor("wqT", [HID, BLK], DT, kind="ExternalInput").ap()
    wkT = nc.dram_tensor("wkT", [HID, BLK], DT, kind="ExternalInput").ap()
    wvT = nc.dram_tensor("wvT", [HID, BLK], DT, kind="ExternalInput").ap()
    woT = nc.dram_tensor("woT", [BLK, HID], DT, kind="ExternalInput").ap()
    cos2 = nc.dram_tensor("cos2", [128, S], DT, kind="ExternalInput").ap()
    sin2s = nc.dram_tensor("sin2s", [128, S], DT, kind="ExternalInput").ap()
    if not causal:
        maskT = nc.dram_tensor("maskT", [S, S], mybir.dt.bfloat16,
                               kind="ExternalInput").ap()
    out = nc.dram_tensor("out", [S, HID], fp32, kind="ExternalOutput").ap()

    tc_ctx = tile.TileContext(nc)

    def phase1(tc, qT_sb, kT_sb, v_sb):
        with tc.tile_pool(name="consts", bufs=1) as cpool, \
             tc.tile_pool(name="wall", bufs=1) as wpool, \
             tc.tile_pool(name="xa", bufs=2) as xa, \
             tc.tile_pool(name="ropetmp", bufs=2) as rt, \
             tc.tile_pool(name="ps1", bufs=8, space="PSUM") as ps1:
            # DMA emission order matters for startup latency: the first
            # matmul group needs x chunk 0 + wq only; cos/sin only matter
            # ~a dozen matmuls later; wk/wv are emitted mid-chunk-0 so
            # they don't steal HBM bandwidth from the critical prefetch.
            wq_sb = wpool.tile([128, 16, BLK], DT)
            wk_sb = wpool.tile([128, 16, BLK], DT)
            wv_sb = wpool.tile([128, 16, BLK], DT)
            cos_sb = cpool.tile([128, S], DT)
            sin_sb = cpool.tile([128, S], DT)

            def dma_w(w_sb, w_d):
                wd = w_d.rearrange("(k p) m -> p k m", p=128)
                for kg in range(4):
                    nc.sync.dma_start(out=w_sb[:, 4 * kg:4 * kg + 4, :],
                                      in_=wd[:, 4 * kg:4 * kg + 4, :])

            x_tiles = {}

            def dma_x(n):
                sl = slice(n * CH, (n + 1) * CH)
                x_ch = xa.tile([128, 16, CH], DT, name=f"x_{n}", tag="x")
                xd = xT[:, sl].rearrange("(k p) t -> p k t", p=128)
                for kg in range(8):
                    nc.sync.dma_start(out=x_ch[:, 2 * kg:2 * kg + 2, :],
                                      in_=xd[:, 2 * kg:2 * kg + 2, :])
                x_tiles[n] = x_ch

            dma_x(0)
            # wq m-tile 0 first: the very first matmul group needs only
            # x chunk 0 + these 512KB, so don't queue the rest ahead of it
            wqd = wqT.rearrange("(k p) m -> p k m", p=128)
            for kg in range(4):
                nc.sync.dma_start(out=wq_sb[:, 4 * kg:4 * kg + 4, 0:128],
                                  in_=wqd[:, 4 * kg:4 * kg + 4, 0:128])
            for kg in range(4):
                nc.sync.dma_start(out=wq_sb[:, 4 * kg:4 * kg + 4, 128:512],
                                  in_=wqd[:, 4 * kg:4 * kg + 4, 128:512])
            nc.sync.dma_start(out=cos_sb, in_=cos2)
            nc.sync.dma_start(out=sin_sb, in_=sin2s)

            for n in range(NCH):
                sl = slice(n * CH, (n + 1) * CH)
                if n not in x_tiles:
                    dma_x(n)
                x_ch = x_tiles.pop(n)
                # --- Q/K + RoPE ---
                for w_sb, dst, wn in ((wq_sb, qT_sb, "q"), (wk_sb, kT_sb, "k")):
                    for m in range(NPR):
                        ps = ps1.tile([128, CH], fp32,
                                      name=f"ps{wn}{m}_{n}", tag="ps1")
                        for k in range(16):
                            nc.tensor.matmul(
                                ps,
                                lhsT=mm(w_sb[:, k, m * 128:(m + 1) * 128]),
                                rhs=mm(x_ch[:, k, :]),
                                start=(k == 0), stop=(k == 15))
                        # rot = partition-shift of ps via 4 ACT copies
                        # (PSUM source, so the SBUF same-start rule only
                        # sees the output), then 2 full-width DVE muls +
                        # GPS add.  Spreads rope across ACT/DVE/GPS.
                        rot = rt.tile([128, CH], fp32, name=f"ro{wn}{m}_{n}", tag="ro")
                        # last chunk's shifts go on DVE so the phase-2 exps
                        # don't queue behind an ACT backlog at the boundary
                        shift_eng = nc.vector if n == NCH - 1 else nc.scalar
                        for (d, s_) in ((0, 32), (32, 0), (64, 96), (96, 64)):
                            if shift_eng is nc.scalar:
                                nc.scalar.copy(rot[d:d + 32, :], ps[s_:s_ + 32, :])
                            else:
                                nc.vector.tensor_copy(rot[d:d + 32, :],
                                                      ps[s_:s_ + 32, :])
                        t1 = rt.tile([128, CH], fp32, name=f"t1{wn}{m}_{n}", tag="t1")
                        t2 = rt.tile([128, CH], fp32, name=f"t2{wn}{m}_{n}", tag="t2")
                        nc.vector.tensor_mul(t1, ps, cos_sb[:, sl])
                        nc.vector.tensor_mul(t2, rot, sin_sb[:, sl])
                        # add on DVE, not GPSIMD: keeping GPSIMD free of
                        # tensor_add means its affine_select + broadcast
                        # libraries stay warm for phase 2
                        nc.vector.tensor_add(dst[:, m, sl], t1, t2)
                    if n == 0 and wn == "q":
                        dma_w(wk_sb, wkT)      # after Q's prefetch drained
                if n == 0:
                    dma_w(wv_sb, wvT)
                    dma_x(1)
                # --- V (same x chunk) ---
                for s_ in range(CH // 128):
                    t16 = (n * CH) // 128 + s_
                    ps = ps1.tile([128, BLK], fp32, name=f"psv{t16}", tag="ps1")
                    for k in range(16):
                        nc.tensor.matmul(
                            ps,
                            lhsT=mm(x_ch[:, k, s_ * 128:(s_ + 1) * 128]),
                            rhs=mm(wv_sb[:, k, :]),
                            start=(k == 0), stop=(k == 15))
                    dstv = v_sb[:, t16, :].rearrange("p (h c) -> p h c", c=65)[:, :, 0:64]
                    nc.vector.tensor_copy(dstv, ps.rearrange("p (h c) -> p h c", c=64))

    with tc_ctx as tc:
        with tc.tile_pool(name="qk_sb", bufs=1) as qk_pool, \
             tc.tile_pool(name="v_sb_pool", bufs=1) as v_pool:
            qT_sb = qk_pool.tile([128, NPR, S], SDT)
            kT_sb = qk_pool.tile([128, NPR, S], SDT)
            v_sb = v_pool.tile([128, NT, HEADS_PER_CORE * 65], SDT)
            ones_ap = v_sb.rearrange("p t (h c) -> p t h c", c=65)[:, :, :, 64:65]
            nc.vector.memset(ones_ap, 1.0)
            # warm the GPSIMD ucode libraries (affine_select + broadcast)
            # during the initial DMA wait — each first use otherwise pays
            # a ~7us library fetch right on the phase-2 critical path.
            # GPSIMD runs nothing else (rope add is on DVE) so these stay
            # resident.
            with tc.tile_pool(name="warm", bufs=1) as warmp:
                wsrc = warmp.tile([1, 64], fp32)
                wdst = warmp.tile([2, 64], fp32)
                nc.vector.memset(wsrc, 1.0)
                nc.gpsimd.affine_select(
                    out=wdst, in_=wdst, compare_op=mybir.AluOpType.is_ge,
                    fill=0.0, base=0, pattern=[[1, 64]], channel_multiplier=0)
                nc.gpsimd.partition_broadcast(wdst, wsrc)

            phase1(tc, qT_sb, kT_sb, v_sb)

            # -------- phase 2+3: attention + fused out-projection ----
            with tc.tile_pool(name="attn_pool", bufs=1) as apool, \
                 tc.tile_pool(name="wo", bufs=1) as wop:
                attn_sb = apool.tile([128, NPR, S], SDT)
                wo_sb = wop.tile([128, NPR, HID], DT)
                nc.sync.dma_start(out=wo_sb,
                                  in_=woT.rearrange("(r p) o -> p r o", p=128))
                with tc.tile_pool(name="ppool", bufs=4) as ppool, \
                     tc.tile_pool(name="npool", bufs=2) as npool, \
                     tc.tile_pool(name="mpool", bufs=2) as mpool, \
                     tc.tile_pool(name="ostage", bufs=2) as ostage, \
                     tc.tile_pool(name="pss", bufs=2, space="PSUM") as pss, \
                     tc.tile_pool(name="psav", bufs=1, space="PSUM") as psav, \
                     tc.tile_pool(name="opool", bufs=2, space="PSUM") as opool:

                    from collections import deque
                    pending_out = deque()

                    def emit_outproj_group(t16, no):
                        o_ps = opool.tile([128, 512], fp32,
                                          name=f"o{t16}_{no}", tag="o")
                        for pr_ in range(NPR):
                            nc.tensor.matmul(
                                o_ps,
                                lhsT=mm(attn_sb[:, pr_,
                                                t16 * 128:(t16 + 1) * 128]),
                                rhs=mm(wo_sb[:, pr_,
                                             no * 512:(no + 1) * 512]),
                                start=(pr_ == 0), stop=(pr_ == NPR - 1))
                        o_sb = ostage.tile([128, 512], fp32,
                                           name=f"os{t16}_{no}", tag="os")
                        nc.vector.tensor_copy(o_sb, o_ps)
                        nc.sync.dma_start(
                            out=out[t16 * 128:(t16 + 1) * 128,
                                    no * 512:(no + 1) * 512],
                            in_=o_sb)

                    def drain_one():
                        if pending_out:
                            t16, no = pending_out.popleft()
                            emit_outproj_group(t16, no)

                    col_iter = [0]

                    def attend(pr, j, mask_col, pace=1):
                        n_i = 4 * j + 4 if causal else NT
                        # full-bank AV tiles: rows 0-64 accumulate
                        # (out.T | denom); rows 64-127 later hold the
                        # PE-broadcast reciprocal
                        av = [psav.tile([128, 512], fp32,
                                        name=f"av{half}_{pr}_{j}", tag=f"av{half}")
                              for half in range(2)]
                        s_t, p_t, nw_t = {}, {}, {}

                        def emit_S(i):
                            # causal boundary tiles only need the tq
                            # suffix [512j+off, 512(j+1))
                            off = max(0, 128 * (i - 4 * j)) if causal else 0
                            nw = 512 - off
                            nw_t[i] = (off, nw)
                            s_ps = pss.tile([128, 1024], fp32,
                                            name=f"s_{pr}_{j}_{i}", tag="s")
                            for half in range(2):
                                r0 = 64 * half
                                nc.tensor.matmul(
                                    s_ps[:, half * 512:half * 512 + nw],
                                    lhsT=mm(kT_sb[r0:r0 + 64, pr,
                                                  i * 128:(i + 1) * 128]),
                                    rhs=mm(qT_sb[r0:r0 + 64, pr,
                                                 j * 512 + off:(j + 1) * 512]),
                                    start=True, stop=True,
                                    tile_position=(r0, 0))
                            s_t[i] = s_ps

                        def emit_exp(i):
                            off, nw = nw_t[i]
                            s_ps = s_t[i]
                            s_v = s_ps.rearrange("q (h t) -> q h t", h=2)[:, :, 0:nw]
                            p = ppool.tile([128, 1024], SDT,
                                           name=f"p_{pr}_{j}_{i}", tag="p")
                            p_v = p.rearrange("q (h t) -> q h t", h=2)[:, :, 0:nw]
                            if causal:
                                nc.scalar.activation(p_v, s_v,
                                                     mybir.ActivationFunctionType.Exp,
                                                     scale=0.125)
                                if i >= 4 * j:
                                    # keep iff tq - tk >= 0 (base is 0 on
                                    # boundary tiles thanks to the suffix)
                                    nc.gpsimd.affine_select(
                                        out=p_v, in_=p_v,
                                        compare_op=mybir.AluOpType.is_ge,
                                        fill=0.0,
                                        base=512 * j + off - 128 * i,
                                        pattern=[[0, 2], [1, nw]],
                                        channel_multiplier=-1)
                            else:
                                tmp = ppool.tile([128, 1024], fp32,
                                                 name=f"pt_{pr}_{j}_{i}", tag="pt")
                                for half in range(2):
                                    nc.vector.scalar_tensor_tensor(
                                        out=tmp[:, half * 512:(half + 1) * 512],
                                        in0=s_ps[:, half * 512:(half + 1) * 512],
                                        scalar=0.125,
                                        in1=mask_col[:, i, :],
                                        op0=mybir.AluOpType.mult,
                                        op1=mybir.AluOpType.add)
                                nc.scalar.activation(p, tmp,
                                                     mybir.ActivationFunctionType.Exp)
                            p_t[i] = p

                        def emit_AV(i, first, last):
                            off, nw = nw_t[i]
                            p = p_t[i]
                            for half in range(2):
                                h = 2 * pr + half
                                nc.tensor.matmul(
                                    av[half][0:65, off:512],
                                    lhsT=mm(v_sb[:, i, 65 * h:65 * h + 65]),
                                    rhs=mm(p[:, half * 512:half * 512 + nw]),
                                    start=first, stop=last)

                        # software pipeline: S runs 2 tiles ahead of AV,
                        # with outproj matmul groups drained between
                        # iterations as PE filler while ACT runs exp.
                        # (ascending i is required: AV i=0 writes the full
                        # 512 with start=True, clearing has_written before
                        # the suffix-trimmed boundary tiles accumulate)
                        order = list(range(n_i))
                        first_i, last_i = order[0], order[-1]
                        emit_S(order[0])
                        emit_exp(order[0])
                        if n_i > 1:
                            emit_S(order[1])
                            emit_exp(order[1])
                        for ii, i in enumerate(order):
                            # drain FIRST: the outproj group is the only PE
                            # work with no dependence on exp(ii) — S needs
                            # exp(ii)'s psum slot and AV needs p(ii) — so it
                            # must go ahead of them in PE's FIFO to cover
                            # the exp tail.  Paced so the queue lasts the
                            # whole column.
                            col_iter[0] += 1
                            if col_iter[0] % pace == 0:
                                drain_one()
                            if ii + 2 < n_i:
                                emit_S(order[ii + 2])
                                emit_exp(order[ii + 2])
                            emit_AV(i, i == first_i, i == last_i)

                        # normalize rows 0..63 by row 64: fast approximate
                        # reciprocal on DVE (~51 ULP, plenty for a softmax
                        # denominator), partition-broadcast on GPSIMD, then
                        # one DVE multiply into attn_sb.  Keeps the PE
                        # stream free of ops that wait on DVE round-trips.
                        # Evict av (rows 0-64) to SBUF first — one copy per
                        # half frees the PSUM slot so the next pair's AV
                        # never waits on the rest of the normalize chain.
                        # Then approx-reciprocal (bitwise custom-DVE op,
                        # needs an SBUF source), GPS partition-broadcast,
                        # and one mul.  u/bcr are full-height tiles so the
                        # two SBUF inputs of the mul share base partition 0.
                        us, recs, bcrs = [], [], []
                        for half in range(2):
                            u = npool.tile([128, 512], fp32,
                                           name=f"u{half}_{pr}_{j}",
                                           tag=f"u{half}")
                            nc.vector.tensor_copy(u[0:65, :], av[half][0:65, :])
                            us.append(u)
                        for half in range(2):
                            # the approx-recip custom-DVE op NaNs when its
                            # input AP starts at a nonzero partition —
                            # bounce the denominator row to partition 0
                            den = npool.tile([1, 512], fp32,
                                             name=f"den{half}_{pr}_{j}",
                                             tag=f"den{half}")
                            nc.vector.tensor_copy(den, us[half][64:65, :])
                            rec = npool.tile([1, 512], fp32,
                                             name=f"rec{half}_{pr}_{j}",
                                             tag=f"rec{half}")
                            nc.vector.reciprocal_approx_fast(rec, den)
                            recs.append(rec)
                        for half in range(2):
                            bcr = npool.tile([128, 512], fp32,
                                             name=f"bc{half}_{pr}_{j}",
                                             tag=f"bc{half}")
                            nc.gpsimd.partition_broadcast(bcr[0:64, :], recs[half])
                            bcrs.append(bcr)
                        for half in range(2):
                            r0 = 64 * half
                            nc.vector.tensor_mul(
                                attn_sb[r0:r0 + 64, pr, j * 512:(j + 1) * 512],
                                us[half][0:64, :], bcrs[half][0:64, :])

                    for j in range(4):
                        if causal:
                            mask_col = None
                        else:
                            mask_col = mpool.tile([128, NT, 512],
                                                  mybir.dt.bfloat16,
                                                  name=f"mc{j}", tag="mc")
                            nc.sync.dma_start(
                                out=mask_col,
                                in_=maskT[:, j * 512:(j + 1) * 512]
                                .rearrange("(i p) t -> p i t", p=128))
                        col_iter[0] = 0
                        for pr in range(NPR):
                            attend(pr, j, mask_col, pace=(j + 1) if causal else 4)
                        # column j's attention rows are complete; queue
                        # its out-projection as PE filler for column j+1
                        for t16 in range(4 * j, 4 * j + 4):
                            for no in range(4):
                                pending_out.append((t16, no))
                    while pending_out:
                        drain_one()
    nc.compile()
    return nc


def _get_program(variant):
    if variant not in _PROGRAMS:
        _PROGRAMS[variant] = build_program(variant)
    return _PROGRAMS[variant]


def make_in_maps(hidden_states, attention_mask, position_ids, Wq, Wk, Wv, Wo,
                 variant):
    npdt = ml_dtypes.bfloat16
    cos, sin = _rope_cache()
    in_maps = []
    for c in range(N_CORES):
        b = c // (N_CORES // 2)
        hb = c % (N_CORES // 2)
        rs = slice(hb * BLK, (hb + 1) * BLK)
        pos = np.asarray(position_ids[b]).astype(np.int64)
        cos_b = cos[pos].T.astype(np.float32)     # [64, S]
        sin_b = sin[pos].T.astype(np.float32)
        sin_s = np.concatenate([-sin_b[:32], sin_b[32:]], axis=0)
        m = {
            "xT": np.ascontiguousarray(np.asarray(hidden_states)[b].T).astype(npdt),
            "wqT": np.ascontiguousarray(np.asarray(Wq)[rs].T).astype(npdt),
            "wkT": np.ascontiguousarray(np.asarray(Wk)[rs].T).astype(npdt),
            "wvT": np.ascontiguousarray(np.asarray(Wv)[rs].T).astype(npdt),
            "woT": np.ascontiguousarray(np.asarray(Wo)[:, rs].T).astype(npdt),
            "cos2": np.ascontiguousarray(
                np.concatenate([cos_b, cos_b], axis=0)).astype(npdt),
            "sin2s": np.ascontiguousarray(
                np.concatenate([sin_s, sin_s], axis=0)).astype(npdt),
        }
        if variant == "general":
            m["maskT"] = np.ascontiguousarray(
                np.asarray(attention_mask)[b, 0].T).astype(ml_dtypes.bfloat16)
        in_maps.append(m)
    return in_maps


def detect_causal(attention_mask):
    am = np.asarray(attention_mask)
    if am.shape != (2, 1, S, S):
        return False
    neg = np.float32(np.finfo(np.float32).min)
    canonical = np.where(np.tril(np.ones((S, S), dtype=bool)), np.float32(0.0), neg)
    return bool(np.array_equal(am[0, 0], canonical) and
                np.array_equal(am[1, 0], canonical))


def kernel(hidden_states, attention_mask, position_ids, Wq, Wk, Wv, Wo):
    hidden_states = np.asarray(hidden_states, dtype=np.float32)
    attention_mask = np.asarray(attention_mask, dtype=np.float32)
    Wq, Wk, Wv, Wo = (np.asarray(w, dtype=np.float32) for w in (Wq, Wk, Wv, Wo))

    variant = "causal" if detect_causal(attention_mask) else "general"
    nc = _get_program(variant)
    in_maps = make_in_maps(hidden_states, attention_mask, position_ids,
                           Wq, Wk, Wv, Wo, variant)

    from concourse import bass2jax
    results = bass2jax.run_bass_via_pjrt(nc, in_maps, n_cores=N_CORES)

    out = np.zeros((2, S, HID), dtype=np.float64)
    for c in range(N_CORES):
        b = c // (N_CORES // 2)
        out[b] += results[c]["out"].astype(np.float64)
    return out.astype(np.float32)


# revision 33
# speedup vs baseline: 1.3318x; 1.0170x over previous
"""MiniCPM attention block on 8 Trainium2 NeuronCores.

Sharding: core c handles batch b = c // 4 and the 8 heads
[ (c%4)*8, (c%4)*8 + 8 ) of that batch (tensor-parallel over heads +
data-parallel over batch).  Each core computes a partial output
x @ block-of-Wo.T of shape [S, HID]; the host sums the 4 partials per
batch.  No collectives.

Device pipeline per core (S=2048 tokens, 8 heads of d=64):
  1. per 512-token chunk: qT/kT = (x @ Wq_s.T).T with RoPE fused into
     the PSUM->SBUF evict, and v = x @ Wv_s.T (stored [tk, 8*65] with a
     ones column per head) — one x load per chunk shared by Q/K/V, all
     12 PSUM groups rotating through one 8-slot pool.
  2. per head-pair, per tq-chunk j (512): S.T tiles [tk 128, tq 512]
     on PE (2 heads packed via row tiling -> concurrent), exp on ACT
     (pair-fused into one [128,1024] op), causal zeroing via gpsimd
     affine_select, AV accumulation on PE with the ones column giving
     the softmax denominator for free.  Normalize via DVE reciprocal +
     a K=1 PE broadcast matmul (ones^T @ rec -> psum rows 64:128) +
     DVE copy/mul — no DRAM bounce.  The i-loop is software-pipelined
     (S two tiles ahead of AV) so PE never waits on the ACT exp
     round-trip.  Causal boundary tiles compute only the valid tq
     suffix.
  3. out_partial = attn_outT.T @ Wo_s.T in [128,512] PSUM groups with
     DVE evicts, interleaved per tq-column so PE has dense work while
     ACT runs the next column's exps.

Self-contained: shapes hardcoded from the problem spec.
"""
import numpy as np
import ml_dtypes

S = 2048
HID = 2048
NH = 32
DH = 64
N_CORES = 8
HEADS_PER_CORE = NH // (N_CORES // 2)   # 8
BLK = HEADS_PER_CORE * DH               # 512
ROPE_BASE = 10000.0

_PROGRAMS = {}


def _rope_cache():
    inv_freq = 1.0 / (ROPE_BASE ** (np.arange(0, DH, 2, dtype=np.float32) / DH))
    t = np.arange(S, dtype=np.float32)
    freqs = np.outer(t, inv_freq)                     # [S, 32]
    emb = np.concatenate([freqs, freqs], axis=-1)     # [S, 64]
    return np.cos(emb), np.sin(emb)


def build_program(variant="causal"):
    """Build the Bacc program (one NEFF, run SPMD on 8 cores)."""
    import concourse.bass as bass
    import concourse.mybir as mybir
    import concourse.tile as tile
    from concourse import bacc

    fp32 = mybir.dt.float32
    f32r = mybir.dt.float32r
    DT = mybir.dt.bfloat16      # dram/lhs/rhs matmul dtype
    SDT = mybir.dt.bfloat16     # on-chip storage for q/k/v/p/attn
    CH = 512                    # token chunk for phase 1

    def mm(ap):
        return ap.bitcast(DT) if DT is not ap.dtype else ap

    causal = variant == "causal"
    NCH = S // CH            # phase-1 token chunks
    NT = S // 128            # 16 token tiles
    NPR = 4                  # head pairs

    nc = bacc.Bacc("TRN2", target_bir_lowering=False, debug=False,
                   enable_asserts=False, num_devices=N_CORES)

    xT = nc.dram_tensor("xT", [HID, S], DT, kind="ExternalInput").ap()
    wqT = nc.dram_tensor("wqT", [HID, BLK], DT, kind="ExternalInput").ap()
    wkT = nc.dram_tensor("wkT", [HID, BLK], DT, kind="ExternalInput").ap()
    wvT = nc.dram_tensor("wvT", [HID, BLK], DT, kind="ExternalInput").ap()
    woT = nc.dram_tensor("woT", [BLK, HID], DT, kind="ExternalInput").ap()
    cos2 = nc.dram_tensor("cos2", [128, S], DT, kind="ExternalInput").ap()
    sin2s = nc.dram_tensor("sin2s", [128, S], DT, kind="ExternalInput").ap()
    if not causal:
        maskT = nc.dram_tensor("maskT", [S, S], mybir.dt.bfloat16,
                               kind="ExternalInput").ap()
    out = nc.dram_tensor("out", [S, HID], fp32, kind="ExternalOutput").ap()

    tc_ctx = tile.TileContext(nc)

    def phase1(tc, qT_sb, kT_sb, v_sb):
        with tc.tile_pool(name="consts", bufs=1) as cpool, \
             tc.tile_pool(name="wall", bufs=1) as wpool, \
             tc.tile_pool(name="xa", bufs=2) as xa, \
             tc.tile_pool(name="ropetmp", bufs=2) as rt, \
             tc.tile_pool(name="ps1", bufs=8, space="PSUM") as ps1:
            # DMA emission order matters for startup latency: the first
            # matmul group needs x chunk 0 + wq only; cos/sin only matter
            # ~a dozen matmuls later; wk/wv are emitted mid-chunk-0 so
            # they don't steal HBM bandwidth from the critical prefetch.
            wq_sb = wpool.tile([128, 16, BLK], DT)
            wk_sb = wpool.tile([128, 16, BLK], DT)
            wv_sb = wpool.tile([128, 16, BLK], DT)
            cos_sb = cpool.tile([128, S], DT)
            sin_sb = cpool.tile([128, S], DT)

            def dma_w(w_sb, w_d):
                wd = w_d.rearrange("(k p) m -> p k m", p=128)
                for kg in range(4):
                    nc.sync.dma_start(out=w_sb[:, 4 * kg:4 * kg + 4, :],
                                      in_=wd[:, 4 * kg:4 * kg + 4, :])

            x_tiles = {}

            def dma_x(n):
                sl = slice(n * CH, (n + 1) * CH)
                x_ch = xa.tile([128, 16, CH], DT, name=f"x_{n}", tag="x")
                xd = xT[:, sl].rearrange("(k p) t -> p k t", p=128)
                for kg in range(8):
                    nc.sync.dma_start(out=x_ch[:, 2 * kg:2 * kg + 2, :],
                                      in_=xd[:, 2 * kg:2 * kg + 2, :])
                x_tiles[n] = x_ch

            dma_x(0)
            # wq m-tile 0 first: the very first matmul group needs only
            # x chunk 0 + these 512KB, so don't queue the rest ahead of it
            wqd = wqT.rearrange("(k p) m -> p k m", p=128)
            for kg in range(4):
                nc.sync.dma_start(out=wq_sb[:, 4 * kg:4 * kg + 4, 0:128],
                                  in_=wqd[:, 4 * kg:4 * kg + 4, 0:128])
            for kg in range(4):
                nc.sync.dma_start(out=wq_sb[:, 4 * kg:4 * kg + 4, 128:512],
                                  in_=wqd[:, 4 * kg:4 * kg + 4, 128:512])
            nc.sync.dma_start(out=cos_sb, in_=cos2)
            nc.sync.dma_start(out=sin_sb, in_=sin2s)

            for n in range(NCH):
                sl = slice(n * CH, (n + 1) * CH)
                if n not in x_tiles:
                    dma_x(n)
                x_ch = x_tiles.pop(n)
                # --- Q/K + RoPE ---
                for w_sb, dst, wn in ((wq_sb, qT_sb, "q"), (wk_sb, kT_sb, "k")):
                    for m in range(NPR):
                        ps = ps1.tile([128, CH], fp32,
                                      name=f"ps{wn}{m}_{n}", tag="ps1")
                        for k in range(16):
                            nc.tensor.matmul(
                                ps,
                                lhsT=mm(w_sb[:, k, m * 128:(m + 1) * 128]),
                                rhs=mm(x_ch[:, k, :]),
                                start=(k == 0), stop=(k == 15))
                        # rot = partition-shift of ps via 4 ACT copies
                        # (PSUM source, so the SBUF same-start rule only
                        # sees the output), then 2 full-width DVE muls +
                        # GPS add.  Spreads rope across ACT/DVE/GPS.
                        rot = rt.tile([128, CH], fp32, name=f"ro{wn}{m}_{n}", tag="ro")
                        # last chunk's shifts go on DVE so the phase-2 exps
                        # don't queue behind an ACT backlog at the boundary
                        shift_eng = nc.vector if n == NCH - 1 else nc.scalar
                        for (d, s_) in ((0, 32), (32, 0), (64, 96), (96, 64)):
                            if shift_eng is nc.scalar:
                                nc.scalar.copy(rot[d:d + 32, :], ps[s_:s_ + 32, :])
                            else:
                                nc.vector.tensor_copy(rot[d:d + 32, :],
                                                      ps[s_:s_ + 32, :])
                        t1 = rt.tile([128, CH], fp32, name=f"t1{wn}{m}_{n}", tag="t1")
                        t2 = rt.tile([128, CH], fp32, name=f"t2{wn}{m}_{n}", tag="t2")
                        nc.vector.tensor_mul(t1, ps, cos_sb[:, sl])
                        nc.vector.tensor_mul(t2, rot, sin_sb[:, sl])
                        # add on DVE, not GPSIMD: keeping GPSIMD free of
                        # tensor_add means its affine_select + broadcast
                        # libraries stay warm for phase 2
                        nc.vector.tensor_add(dst[:, m, sl], t1, t2)
                    if n == 0 and wn == "q":
                        dma_w(wk_sb, wkT)      # after Q's prefetch drained
                if n == 0:
                    dma_w(wv_sb, wvT)
                    dma_x(1)
                # --- V (same x chunk) ---
                for s_ in range(CH // 128):
                    t16 = (n * CH) // 128 + s_
                    ps = ps1.tile([128, BLK], fp32, name=f"psv{t16}", tag="ps1")
                    for k in range(16):
                        nc.tensor.matmul(
                            ps,
                            lhsT=mm(x_ch[:, k, s_ * 128:(s_ + 1) * 128]),
                            rhs=mm(wv_sb[:, k, :]),
                            start=(k == 0), stop=(k == 15))
                    dstv = v_sb[:, t16, :].rearrange("p (h c) -> p h c", c=65)[:, :, 0:64]
                    nc.vector.tensor_copy(dstv, ps.rearrange("p (h c) -> p h c", c=64))

    with tc_ctx as tc:
        with tc.tile_pool(name="qk_sb", bufs=1) as qk_pool, \
             tc.tile_pool(name="v_sb_pool", bufs=1) as v_pool:
            qT_sb = qk_pool.tile([128, NPR, S], SDT)
            kT_sb = qk_pool.tile([128, NPR, S], SDT)
            v_sb = v_pool.tile([128, NT, HEADS_PER_CORE * 65], SDT)
            ones_ap = v_sb.rearrange("p t (h c) -> p t h c", c=65)[:, :, :, 64:65]
            nc.vector.memset(ones_ap, 1.0)
            # warm the GPSIMD ucode libraries (affine_select + broadcast)
            # during the initial DMA wait — each first use otherwise pays
            # a ~7us library fetch right on the phase-2 critical path.
            # GPSIMD runs nothing else (rope add is on DVE) so these stay
            # resident.
            with tc.tile_pool(name="warm", bufs=1) as warmp:
                wsrc = warmp.tile([1, 64], fp32)
                wdst = warmp.tile([2, 64], fp32)
                nc.vector.memset(wsrc, 1.0)
                nc.gpsimd.affine_select(
                    out=wdst, in_=wdst, compare_op=mybir.AluOpType.is_ge,
                    fill=0.0, base=0, pattern=[[1, 64]], channel_multiplier=0)
                nc.gpsimd.partition_broadcast(wdst, wsrc)

            phase1(tc, qT_sb, kT_sb, v_sb)

            # -------- phase 2+3: attention + fused out-projection ----
            with tc.tile_pool(name="attn_pool", bufs=1) as apool, \
                 tc.tile_pool(name="wo", bufs=1) as wop:
                attn_sb = apool.tile([128, NPR, S], SDT)
                wo_sb = wop.tile([128, NPR, HID], DT)
                nc.sync.dma_start(out=wo_sb,
                                  in_=woT.rearrange("(r p) o -> p r o", p=128))
                with tc.tile_pool(name="ppool", bufs=4) as ppool, \
                     tc.tile_pool(name="npool", bufs=2) as npool, \
                     tc.tile_pool(name="mpool", bufs=2) as mpool, \
                     tc.tile_pool(name="ostage", bufs=2) as ostage, \
                     tc.tile_pool(name="pss", bufs=2, space="PSUM") as pss, \
                     tc.tile_pool(name="psav", bufs=1, space="PSUM") as psav, \
                     tc.tile_pool(name="opool", bufs=2, space="PSUM") as opool:

                    from collections import deque
                    pending_out = deque()

                    def emit_outproj_group(t16, no):
                        o_ps = opool.tile([128, 512], fp32,
                                          name=f"o{t16}_{no}", tag="o")
                        for pr_ in range(NPR):
                            nc.tensor.matmul(
                                o_ps,
                                lhsT=mm(attn_sb[:, pr_,
                                                t16 * 128:(t16 + 1) * 128]),
                                rhs=mm(wo_sb[:, pr_,
                                             no * 512:(no + 1) * 512]),
                                start=(pr_ == 0), stop=(pr_ == NPR - 1))
                        o_sb = ostage.tile([128, 512], fp32,
                                           name=f"os{t16}_{no}", tag="os")
                        nc.vector.tensor_copy(o_sb, o_ps)
                        nc.sync.dma_start(
                            out=out[t16 * 128:(t16 + 1) * 128,
                                    no * 512:(no + 1) * 512],
                            in_=o_sb)

                    def drain_one():
                        if pending_out:
                            t16, no = pending_out.popleft()
                            emit_outproj_group(t16, no)

                    col_iter = [0]

                    def attend(pr, j, mask_col, pace=1):
                        n_i = 4 * j + 4 if causal else NT
                        # full-bank AV tiles: rows 0-64 accumulate
                        # (out.T | denom); rows 64-127 later hold the
                        # PE-broadcast reciprocal
                        av = [psav.tile([128, 512], fp32,
                                        name=f"av{half}_{pr}_{j}", tag=f"av{half}")
                              for half in range(2)]
                        s_t, p_t, nw_t = {}, {}, {}

                        def emit_S(i):
                            # causal boundary tiles only need the tq
                            # suffix [512j+off, 512(j+1))
                            off = max(0, 128 * (i - 4 * j)) if causal else 0
                            nw = 512 - off
                            nw_t[i] = (off, nw)
                            s_ps = pss.tile([128, 1024], fp32,
                                            name=f"s_{pr}_{j}_{i}", tag="s")
                            for half in range(2):
                                r0 = 64 * half
                                nc.tensor.matmul(
                                    s_ps[:, half * 512:half * 512 + nw],
                                    lhsT=mm(kT_sb[r0:r0 + 64, pr,
                                                  i * 128:(i + 1) * 128]),
                                    rhs=mm(qT_sb[r0:r0 + 64, pr,
                                                 j * 512 + off:(j + 1) * 512]),
                                    start=True, stop=True,
                                    tile_position=(r0, 0))
                            s_t[i] = s_ps

                        def emit_exp(i):
                            off, nw = nw_t[i]
                            s_ps = s_t[i]
                            s_v = s_ps.rearrange("q (h t) -> q h t", h=2)[:, :, 0:nw]
                            p = ppool.tile([128, 1024], SDT,
                                           name=f"p_{pr}_{j}_{i}", tag="p")
                            p_v = p.rearrange("q (h t) -> q h t", h=2)[:, :, 0:nw]
                            if causal:
                                nc.scalar.activation(p_v, s_v,
                                                     mybir.ActivationFunctionType.Exp,
                                                     scale=0.125)
                                if i >= 4 * j:
                                    # keep iff tq - tk >= 0 (base is 0 on
                                    # boundary tiles thanks to the suffix).
                                    # Only the first 128 suffix columns can
                                    # violate causality (beyond them
                                    # tq' >= 128 > tk always), so mask just
                                    # that block — the rest of the tile's
                                    # AV then never waits on GPSIMD.
                                    mw = min(nw, 128)
                                    nc.gpsimd.affine_select(
                                        out=p_v[:, :, 0:mw],
                                        in_=p_v[:, :, 0:mw],
                                        compare_op=mybir.AluOpType.is_ge,
                                        fill=0.0,
                                        base=512 * j + off - 128 * i,
                                        pattern=[[0, 2], [1, mw]],
                                        channel_multiplier=-1)
                            else:
                                tmp = ppool.tile([128, 1024], fp32,
                                                 name=f"pt_{pr}_{j}_{i}", tag="pt")
                                for half in range(2):
                                    nc.vector.scalar_tensor_tensor(
                                        out=tmp[:, half * 512:(half + 1) * 512],
                                        in0=s_ps[:, half * 512:(half + 1) * 512],
                                        scalar=0.125,
                                        in1=mask_col[:, i, :],
                                        op0=mybir.AluOpType.mult,
                                        op1=mybir.AluOpType.add)
                                nc.scalar.activation(p, tmp,
                                                     mybir.ActivationFunctionType.Exp)
                            p_t[i] = p

                        def emit_AV(i, first, last):
                            off, nw = nw_t[i]
                            p = p_t[i]
                            boundary = causal and i >= 4 * j
                            for half in range(2):
                                h = 2 * pr + half
                                vt = mm(v_sb[:, i, 65 * h:65 * h + 65])
                                if boundary and nw > 128:
                                    # split at the masked block: columns
                                    # past it depend only on exp, not the
                                    # GPS select.  start=True clears the
                                    # bank's has_written marks, so when
                                    # this is the group opener the LAST
                                    # start=True matmul must be the one
                                    # whose columns later tiles accumulate
                                    # into (>=128); accumulating tiles are
                                    # pure adds and take the AS-decoupled
                                    # order instead.
                                    masked = (av[half][0:65, off:off + 128],
                                              mm(p[:, half * 512:
                                                   half * 512 + 128]))
                                    rest = (av[half][0:65, off + 128:512],
                                            mm(p[:, half * 512 + 128:
                                                 half * 512 + nw]))
                                    pair = (masked, rest) if first \
                                        else (rest, masked)
                                    nc.tensor.matmul(
                                        pair[0][0], lhsT=vt, rhs=pair[0][1],
                                        start=first, stop=False)
                                    nc.tensor.matmul(
                                        pair[1][0], lhsT=vt, rhs=pair[1][1],
                                        start=first, stop=last)
                                else:
                                    nc.tensor.matmul(
                                        av[half][0:65, off:512],
                                        lhsT=vt,
                                        rhs=mm(p[:, half * 512:
                                                 half * 512 + nw]),
                                        start=first, stop=last)

                        # software pipeline: S runs 2 tiles ahead of AV,
                        # with outproj matmul groups drained between
                        # iterations as PE filler while ACT runs exp.
                        # (ascending i is required: AV i=0 writes the full
                        # 512 with start=True, clearing has_written before
                        # the suffix-trimmed boundary tiles accumulate)
                        order = list(range(n_i))
                        first_i, last_i = order[0], order[-1]
                        emit_S(order[0])
                        emit_exp(order[0])
                        if n_i > 1:
                            emit_S(order[1])
                            emit_exp(order[1])
                        for ii, i in enumerate(order):
                            # drain FIRST: the outproj group is the only PE
                            # work with no dependence on exp(ii) — S needs
                            # exp(ii)'s psum slot and AV needs p(ii) — so it
                            # must go ahead of them in PE's FIFO to cover
                            # the exp tail.  Paced so the queue lasts the
                            # whole column.
                            col_iter[0] += 1
                            if col_iter[0] % pace == 0:
                                drain_one()
                            if ii + 2 < n_i:
                                emit_S(order[ii + 2])
                                emit_exp(order[ii + 2])
                            emit_AV(i, i == first_i, i == last_i)

                        # normalize rows 0..63 by row 64: fast approximate
                        # reciprocal on DVE (~51 ULP, plenty for a softmax
                        # denominator), partition-broadcast on GPSIMD, then
                        # one DVE multiply into attn_sb.  Keeps the PE
                        # stream free of ops that wait on DVE round-trips.
                        # Evict av (rows 0-64) to SBUF first — one copy per
                        # half frees the PSUM slot so the next pair's AV
                        # never waits on the rest of the normalize chain.
                        # Then approx-reciprocal (bitwise custom-DVE op,
                        # needs an SBUF source), GPS partition-broadcast,
                        # and one mul.  u/bcr are full-height tiles so the
                        # two SBUF inputs of the mul share base partition 0.
                        us, recs, bcrs = [], [], []
                        for half in range(2):
                            u = npool.tile([128, 512], fp32,
                                           name=f"u{half}_{pr}_{j}",
                                           tag=f"u{half}")
                            nc.vector.tensor_copy(u[0:65, :], av[half][0:65, :])
                            us.append(u)
                        for half in range(2):
                            # the approx-recip custom-DVE op NaNs when its
                            # input AP starts at a nonzero partition —
                            # bounce the denominator row to partition 0
                            den = npool.tile([1, 512], fp32,
                                             name=f"den{half}_{pr}_{j}",
                                             tag=f"den{half}")
                            nc.vector.tensor_copy(den, us[half][64:65, :])
                            rec = npool.tile([1, 512], fp32,
                                             name=f"rec{half}_{pr}_{j}",
                                             tag=f"rec{half}")
                            nc.vector.reciprocal_approx_fast(rec, den)
                            recs.append(rec)
                        for half in range(2):
                            bcr = npool.tile([128, 512], fp32,
                                             name=f"bc{half}_{pr}_{j}",
                                             tag=f"bc{half}")
                            nc.gpsimd.partition_broadcast(bcr[0:64, :], recs[half])
                            bcrs.append(bcr)
                        for half in range(2):
                            r0 = 64 * half
                            nc.vector.tensor_mul(
                                attn_sb[r0:r0 + 64, pr, j * 512:(j + 1) * 512],
                                us[half][0:64, :], bcrs[half][0:64, :])

                    for j in range(4):
                        if causal:
                            mask_col = None
                        else:
                            mask_col = mpool.tile([128, NT, 512],
                                                  mybir.dt.bfloat16,
                                                  name=f"mc{j}", tag="mc")
                            nc.sync.dma_start(
                                out=mask_col,
                                in_=maskT[:, j * 512:(j + 1) * 512]
                                .rearrange("(i p) t -> p i t", p=128))
                        col_iter[0] = 0
                        for pr in range(NPR):
                            attend(pr, j, mask_col, pace=(j + 1) if causal else 4)
                        # column j's attention rows are complete; queue
                        # its out-projection as PE filler for column j+1
                        for t16 in range(4 * j, 4 * j + 4):
                            for no in range(4):
                                pending_out.append((t16, no))
                    while pending_out:
                        drain_one()
    nc.compile()
    return nc


def _get_program(variant):
    if variant not in _PROGRAMS:
        _PROGRAMS[variant] = build_program(variant)
    return _PROGRAMS[variant]


def make_in_maps(hidden_states, attention_mask, position_ids, Wq, Wk, Wv, Wo,
                 variant):
    npdt = ml_dtypes.bfloat16
    cos, sin = _rope_cache()
    in_maps = []
    for c in range(N_CORES):
        b = c // (N_CORES // 2)
        hb = c % (N_CORES // 2)
        rs = slice(hb * BLK, (hb + 1) * BLK)
        pos = np.asarray(position_ids[b]).astype(np.int64)
        cos_b = cos[pos].T.astype(np.float32)     # [64, S]
        sin_b = sin[pos].T.astype(np.float32)
        sin_s = np.concatenate([-sin_b[:32], sin_b[32:]], axis=0)
        m = {
            "xT": np.ascontiguousarray(np.asarray(hidden_states)[b].T).astype(npdt),
            "wqT": np.ascontiguousarray(np.asarray(Wq)[rs].T).astype(npdt),
            "wkT": np.ascontiguousarray(np.asarray(Wk)[rs].T).astype(npdt),
            "wvT": np.ascontiguousarray(np.asarray(Wv)[rs].T).astype(npdt),
            "woT": np.ascontiguousarray(np.asarray(Wo)[:, rs].T).astype(npdt),
            "cos2": np.ascontiguousarray(
                np.concatenate([cos_b, cos_b], axis=0)).astype(npdt),
            "sin2s": np.ascontiguousarray(
                np.concatenate([sin_s, sin_s], axis=0)).astype(npdt),
        }
        if variant == "general":
            m["maskT"] = np.ascontiguousarray(
                np.asarray(attention_mask)[b, 0].T).astype(ml_dtypes.bfloat16)
        in_maps.append(m)
    return in_maps


def detect_causal(attention_mask):
    am = np.asarray(attention_mask)
    if am.shape != (2, 1, S, S):
        return False
    neg = np.float32(np.finfo(np.float32).min)
    canonical = np.where(np.tril(np.ones((S, S), dtype=bool)), np.float32(0.0), neg)
    return bool(np.array_equal(am[0, 0], canonical) and
                np.array_equal(am[1, 0], canonical))


def kernel(hidden_states, attention_mask, position_ids, Wq, Wk, Wv, Wo):
    hidden_states = np.asarray(hidden_states, dtype=np.float32)
    attention_mask = np.asarray(attention_mask, dtype=np.float32)
    Wq, Wk, Wv, Wo = (np.asarray(w, dtype=np.float32) for w in (Wq, Wk, Wv, Wo))

    variant = "causal" if detect_causal(attention_mask) else "general"
    nc = _get_program(variant)
    in_maps = make_in_maps(hidden_states, attention_mask, position_ids,
                           Wq, Wk, Wv, Wo, variant)

    from concourse import bass2jax
    results = bass2jax.run_bass_via_pjrt(nc, in_maps, n_cores=N_CORES)

    out = np.zeros((2, S, HID), dtype=np.float64)
    for c in range(N_CORES):
        b = c // (N_CORES // 2)
        out[b] += results[c]["out"].astype(np.float64)
    return out.astype(np.float32)
